# revision 2
# baseline (speedup 1.0000x reference)
"""Multi-head attention TRN2 kernel, v2.

Full inputs -> 8-core shard (batch x head-group) -> Bass/Tile kernel -> host
gather+reduce.  Problem: B=2, S=2048, D=1024, H=16, Dk=64, fp32, mask=0.

Core c = b*4 + g handles batch b, heads 4g..4g+3.  All intermediates fp16.

Engine plan (cost-model driven):
  PE    projections (full-128 contraction), scoresT (K=64), ctx with the
        attn chunk STATIONARY and V MOVING (65 rows incl. a ones column so
        the softmax denominator rides along), Wo.
  Act   exp only: one [128,1024] activation per iteration (two j-chunks
        side by side), scale=0.125 folded in.  This stream (~134us busy)
        is the bottleneck; everything else hides under it.
  DVE   psum->sbuf copies + reciprocals.
  Pool  softmax normalize + half the psum->sbuf drains.
  DMA   input stream, ctx transposes via the XBAR, output writes.

Sweeps (i-quarter q, head h) are ordered h-pair-outer: heads 0/1 for all
quarters first (sweeps 0..7 = q0h0,q0h1,q1h0,...), then heads 2/3
(sweeps 8..15).  KT/QT chunk-0 thus feeds the first 8 sweeps and chunk-1
streams in later.  Sweeps 0 and 1 are jc-interleaved into one 16-slot
stretch so the fresh-KT demand rate stays under the HBM stream rate.
ctx matmuls run a few iterations behind their exp (explicit CIT map);
V/QT/KT/Wo pieces fill PE slack via a deadline-greedy budgeter.
PSUM: scores ring 2x2 banks, ctx accums 2x1 bank, wo/proj ring 2x1 bank.
"""

import sys

import numpy as np

try:
    import concourse.bass as bass  # noqa: F401
except ImportError:  # harness runs from a bare directory
    sys.path.insert(0, "/opt/trn_rl_repo")
    import concourse.bass as bass  # noqa: F401

import concourse.tile as tile
from concourse import bacc, mybir
from concourse.bass_utils import run_bass_kernel_spmd

S = 2048
D = 1024
HG = 4  # heads per core
DK = 64
DKV = HG * DK  # 256
P = 128
F32 = mybir.dt.float32
FP16 = mybir.dt.float16
EXP = mybir.ActivationFunctionType.Exp

_NC_CACHE = []

AT_RING = 15
CAP = 2491  # PE cycles per Act period (1038ns at 2.4GHz)
MAX_DEBT = 1200  # PE-behind allowance absorbed by the sc ring


def _iteration_maps():
    """SIT: (s, i2) -> scores iteration; CIT: ctx iteration; both 0-based
    over 128 slots.  Sweeps 0/1 are interleaved over slots 0..15."""
    sit = {}
    for s in range(16):
        for i2 in range(8):
            if s < 2:
                sit[(s, i2)] = 2 * i2 + s
            else:
                sit[(s, i2)] = 8 * s + i2

    lag = {0: 12, 1: 12, 2: 12, 3: 11, 4: 10, 5: 9, 6: 8, 7: 7,
           8: 6, 9: 5, 10: 4, 11: 3, 12: 2, 13: 2, 14: 2, 15: 1}
    cit = {}
    for s in range(16):
        L = lag.get(s, 2)
        for i2 in range(8):
            cit[(s, i2)] = sit[(s, i2)] + L
    return sit, cit


def _build_nc():
    nc = bacc.Bacc("TRN2", target_bir_lowering=False, debug=False)
    xqT = nc.dram_tensor("xqT", [D, S], FP16, kind="ExternalInput")
    xkT = nc.dram_tensor("xkT", [D, S], FP16, kind="ExternalInput")
    xvT = nc.dram_tensor("xvT", [D, S], FP16, kind="ExternalInput")
    wq = nc.dram_tensor("wq", [D, DKV], FP16, kind="ExternalInput")
    wk = nc.dram_tensor("wk", [D, DKV], FP16, kind="ExternalInput")
    wv = nc.dram_tensor("wv", [D, DKV], FP16, kind="ExternalInput")
    wo = nc.dram_tensor("wo", [DKV, D], FP16, kind="ExternalInput")
    idn = nc.dram_tensor("idn", [P, P], FP16, kind="ExternalInput")
    out = nc.dram_tensor("out", [S, D], F32, kind="ExternalOutput")

    sit, cit = _iteration_maps()
    scores_at = {}  # iter -> (s, i2)
    for k, t in sit.items():
        scores_at[t] = k
    ctx_at = {}
    for (s, i2), t in cit.items():
        ctx_at.setdefault(t, []).append((s, i2))
        ctx_at[t].sort(key=lambda k: (cit[k], k))
    norm_after = {}  # (s, i2) -> s to normalize right after that ctx
    for s in range(16):
        norm_after[(s, 7)] = s

    with tile.TileContext(nc) as tc:
        with (
            tc.tile_pool(name="persist", bufs=1) as persist,
            tc.tile_pool(name="xk_p", bufs=8) as xk_p,
            tc.tile_pool(name="xq_p", bufs=8) as xq_p,
            tc.tile_pool(name="xv_p", bufs=4) as xv_p,
            tc.tile_pool(name="at_p", bufs=AT_RING) as at_p,
            tc.tile_pool(name="stage", bufs=2) as stage_p,
            tc.tile_pool(name="q3st", bufs=4) as q3st_p,
            tc.tile_pool(name="small", bufs=4) as small_p,
            tc.tile_pool(name="dmy", bufs=1) as dmy_p,
            tc.tile_pool(name="sc_ps", bufs=2, space="PSUM") as sc_ps,
            tc.tile_pool(name="cx_ps", bufs=2, space="PSUM") as cx_ps,
            tc.tile_pool(name="wo_ps", bufs=2, space="PSUM") as wo_ps,
        ):
            # ---- persistent SBUF ----
            QT = persist.tile([P, 2, S], FP16)  # [(h%2)*64+dk, h//2, i]
            KT = persist.tile([P, 2, S], FP16)
            Vag = persist.tile([P, HG, 16, DK + 1], FP16)  # [j%128, h, jc, dk|1]
            ctx_sb = persist.tile([P, 16, DKV], FP16)  # [i%128, ib, dkv]
            ctxT = persist.tile([P, 2, S], FP16)  # [dkv%128, dkv//128, i]
            wq_sb = persist.tile([P, 8, DKV], FP16)
            wk_sb = persist.tile([P, 8, DKV], FP16)
            wv_sb = persist.tile([P, 8, DKV], FP16)
            wo_sb = persist.tile([P, 2, D], FP16)
            idn_sb = persist.tile([P, P], FP16)

            # ---- head: exp-table warm + ones column ----
            warm = small_p.tile([P, 8], F32, tag="warm")
            nc.vector.memset(warm[0:1, :], 0.0)
            nc.scalar.activation(warm[0:1, :], warm[0:1, :], EXP, scale=0.0)
            nc.vector.memset(Vag[:, :, :, DK : DK + 1], 1.0)

            xk_t, xq_t, xv_t = {}, {}, {}

            def load_x(pool, store, xT, idx, width, name):
                t = pool.tile([P, 8, width], FP16, tag="x", name=f"{name}{idx}")
                nc.sync.dma_start(
                    t,
                    xT.rearrange("(ko p) i -> p ko i", p=P)[
                        :, :, idx * width : (idx + 1) * width
                    ],
                )
                store[idx] = t

            # DMA issue order = arrival priority (in-order SP queue).
            nc.sync.dma_start(wk_sb, wk.rearrange("(ko p) n -> p ko n", p=P))
            load_x(xk_p, xk_t, xkT, 0, 256, "xk")
            nc.sync.dma_start(wq_sb, wq.rearrange("(ko p) n -> p ko n", p=P))
            load_x(xq_p, xq_t, xqT, 0, 256, "xq")
            load_x(xq_p, xq_t, xqT, 1, 256, "xq")
            for e in range(1, 8):
                load_x(xk_p, xk_t, xkT, e, 256, "xk")
            nc.sync.dma_start(wv_sb, wv.rearrange("(ko p) n -> p ko n", p=P))
            load_x(xv_p, xv_t, xvT, 0, 512, "xv")
            load_x(xq_p, xq_t, xqT, 2, 256, "xq")
            load_x(xq_p, xq_t, xqT, 3, 256, "xq")
            load_x(xv_p, xv_t, xvT, 1, 512, "xv")
            load_x(xv_p, xv_t, xvT, 2, 512, "xv")
            load_x(xv_p, xv_t, xvT, 3, 512, "xv")
            load_x(xq_p, xq_t, xqT, 4, 256, "xq")
            load_x(xq_p, xq_t, xqT, 5, 256, "xq")
            load_x(xq_p, xq_t, xqT, 6, 256, "xq")
            load_x(xq_p, xq_t, xqT, 7, 256, "xq")
            nc.sync.dma_start(wo_sb, wo.rearrange("(c p) n -> p c n", p=P))
            nc.sync.dma_start(idn_sb, idn[:, :])

            # ---- piece emitters (all 256-wide sub-pieces for KT/QT) ----
            qk_open = {}

            def qk_sub(w_sb, x_t, OUT, c, e, part=2):
                """OUT[:, c, e*256:(e+1)*256] from x eighth e (2048 cycles);
                part=0/1 emit one 1024-cycle half of the k-accumulation."""
                key = (id(w_sb), c, e)
                if part in (0, 2):
                    qk_open[key] = wo_ps.tile([P, 512], F32, tag="wo", name="qk")
                ps = qk_open[key]
                ks = range(8) if part == 2 else range(4 * part, 4 * part + 4)
                for k in ks:
                    nc.tensor.matmul(
                        ps[:, 0:256],
                        w_sb[:, k, c * P : (c + 1) * P],
                        x_t[e][:, k, :],
                        start=(k == 0),
                        stop=(k == 7),
                    )
                if part in (1, 2):
                    nc.vector.tensor_copy(
                        OUT[:, c, e * 256 : (e + 1) * 256], ps[:, 0:256]
                    )
                    del qk_open[key]

            def v_piece(jc, half):
                """Vag[:, 2*half:2*half+2, jc, 0:64] (1024 cycles)."""
                ps = wo_ps.tile([P, 512], F32, tag="wo", name="v")
                xt = xv_t[jc // 4]
                j0 = (jc % 4) * P
                for k in range(8):
                    nc.tensor.matmul(
                        ps[:, 0:P],
                        xt[:, k, j0 : j0 + P],
                        wv_sb[:, k, half * P : (half + 1) * P],
                        start=(k == 0),
                        stop=(k == 7),
                    )
                nc.vector.tensor_copy(
                    Vag[:, 2 * half : 2 * half + 2, jc, 0:DK],
                    ps[:, 0:P].rearrange("p (h d) -> p h d", h=2),
                )

            stages = {}
            wo_done = {}

            def wo_piece(ib, eh):
                if ib not in stages:
                    stages[ib] = stage_p.tile([P, D], F32, tag="st", name=f"st{ib}")
                ps = wo_ps.tile([P, 512], F32, tag="wo", name="woo")
                for c in range(2):
                    nc.tensor.matmul(
                        ps,
                        ctxT[:, c, ib * P : (ib + 1) * P],
                        wo_sb[:, c, eh * 512 : (eh + 1) * 512],
                        start=(c == 0),
                        stop=(c == 1),
                    )
                st = stages[ib]
                eng = nc.vector
                eng.tensor_copy(st[:, eh * 512 : (eh + 1) * 512], ps)
                done = wo_done.setdefault(ib, set())
                done.add(eh)
                if done == {0, 1}:
                    nc.sync.dma_start(out[ib * P : (ib + 1) * P, :], st)

            # Last quarter: wo split into an early c0 half (staged to SBUF)
            # and a small tail c1 half + add, to shorten the tail chain.
            def wo_c0_piece(ib, eh):
                if ib not in stages:
                    stages[ib] = q3st_p.tile([P, D], F32, tag="q3st",
                                             name=f"q3st{ib}")
                ps = wo_ps.tile([P, 512], F32, tag="wo", name="woa")
                nc.tensor.matmul(
                    ps,
                    ctxT[:, 0, ib * P : (ib + 1) * P],
                    wo_sb[:, 0, eh * 512 : (eh + 1) * 512],
                    start=True,
                    stop=True,
                )
                eng = nc.vector
                eng.tensor_copy(stages[ib][:, eh * 512 : (eh + 1) * 512], ps)

            wo_c1_ps = {}

            def wo_c1_piece(ib, eh):
                if ib not in wo_c1_ps:
                    wo_c1_ps[ib] = sc_ps.tile([P, 1024], F32, tag="sc", name="wob")
                ps = wo_c1_ps[ib][:, eh * 512 : (eh + 1) * 512]
                nc.tensor.matmul(
                    ps,
                    ctxT[:, 1, ib * P : (ib + 1) * P],
                    wo_sb[:, 1, eh * 512 : (eh + 1) * 512],
                    start=True,
                    stop=True,
                )
                st = stages[ib]
                sl = st[:, eh * 512 : (eh + 1) * 512]
                eng = nc.vector
                eng.scalar_tensor_tensor(
                    sl, ps, 1.0, sl, mybir.AluOpType.mult, mybir.AluOpType.add
                )
                nc.sync.dma_start(
                    out[ib * P : (ib + 1) * P, eh * 512 : (eh + 1) * 512], sl
                )

            cx_tiles = {}
            at_tiles = {}

            def scores_exp(s, i2):
                q, h = (s % 8) // 2, 2 * (s // 8) + (s % 2)
                c, r0 = h // 2, (h % 2) * DK
                sc = sc_ps.tile([P, 1024], F32, tag="sc", name="sc")
                for jj in range(2):
                    jc = 2 * i2 + jj
                    nc.tensor.matmul(
                        sc[:, jj * 512 : (jj + 1) * 512],
                        KT[r0 : r0 + DK, c, jc * P : (jc + 1) * P],
                        QT[r0 : r0 + DK, c, q * 512 : (q + 1) * 512],
                        start=True,
                        stop=True,
                    )
                at = at_p.tile([P, 1024], FP16, tag="at", name=f"at{s}_{i2}")
                nc.scalar.activation(at, sc, EXP, scale=0.125)
                at_tiles[(s, i2)] = at

            def ctx_mm(s, i2):
                q, h = (s % 8) // 2, 2 * (s // 8) + (s % 2)
                at = at_tiles.pop((s, i2))
                cx = cx_tiles[s]
                for jj in range(2):
                    jc = 2 * i2 + jj
                    for ib in range(4):
                        nc.tensor.matmul(
                            cx[:, ib, :],
                            at[:, jj * 512 + ib * P : jj * 512 + (ib + 1) * P],
                            Vag[:, h, jc, :],
                            start=False,
                            stop=(i2 == 7 and jj == 1),
                            skip_group_check=True,
                        )

            def norm_sweep(s):
                q, h = (s % 8) // 2, 2 * (s // 8) + (s % 2)
                cx = cx_tiles.pop(s)
                second = h % 2 == 1  # both heads of chunk h//2 done
                rc = small_p.tile([P, 4], F32, tag="rc")
                with nc.allow_low_precision("softmax denom"):
                    nc.vector.reciprocal(
                        rc, cx[:, :, DK : DK + 1].rearrange("p a b -> p (a b)")
                    )
                for ib in range(4):
                    nc.vector.tensor_scalar_mul(
                        ctx_sb[:, q * 4 + ib, h * DK : (h + 1) * DK],
                        cx[:, ib, 0:DK],
                        rc[:, ib : ib + 1],
                    )
                    if second and s != 15:
                        nc.sync.dma_start_transpose(
                            ctxT[:, h // 2, (q * 4 + ib) * P : (q * 4 + ib + 1) * P],
                            ctx_sb[:, q * 4 + ib, (h // 2) * P : (h // 2 + 1) * P],
                        )
                if s == 15:  # tail: PE transposes (no HWDGE serialization)
                    for ib in range(4):
                        tp = wo_ps.tile([P, P], FP16, tag="wo", name="tp")
                        nc.tensor.transpose(
                            tp, ctx_sb[:, 12 + ib, P : 2 * P], idn_sb
                        )
                        eng = nc.vector
                        eng.tensor_copy(
                            ctxT[:, 1, (12 + ib) * P : (13 + ib) * P], tp
                        )

            # ---- weave units: (deadline, ready, cycles, fn) ----
            weaves = []
            # KT c0 eighths 1..7 (e0 in head): deadline iter 2e (pair-0).
            kt0_rdy = [0, 1, 2, 3, 5, 6, 8, 9]
            for e in range(1, 8):
                for part in range(2):
                    weaves.append(
                        (2 * e - 1, kt0_rdy[e], 1024,
                         lambda e=e, part=part: qk_sub(wk_sb, xk_t, KT, 0, e, part))
                    )
            # KT c1 eighths: needed from iter 64; weave in 20..60.
            for e in range(8):
                for part in range(2):
                    weaves.append(
                        (40 + 2 * e, 14 + e, 1024,
                         lambda e=e, part=part: qk_sub(wk_sb, xk_t, KT, 1, e, part))
                    )
            # QT c0 eighths 2..7 (e0,e1 in head): (c0, qi) by iter 16*qi.
            for e in range(2, 8):
                qi = e // 2
                for part in range(2):
                    weaves.append(
                        (16 * qi - 1, {2: 13, 3: 14, 4: 25, 5: 26, 6: 27, 7: 29}[e],
                         1024, lambda e=e, part=part: qk_sub(wq_sb, xq_t, QT, 0, e, part))
                    )
            # QT c1 eighths: by iter 64 + 16*qi; weave 22..60.
            for e in range(8):
                qi = e // 2
                for part in range(2):
                    weaves.append(
                        (63 + 16 * qi, 22 + e if e < 4 else 25 + e, 1024,
                         lambda e=e, part=part: qk_sub(wq_sb, xq_t, QT, 1, e, part))
                    )
            # V half-0 (heads 0,1): V[jc] by ctx of pair-0 at iter 2*(jc//2)+12.
            for jc in range(16):
                rdy = [12, 18, 21, 23][jc // 4]
                weaves.append(
                    (max(2 * (jc // 2) + 11, rdy), rdy, 1024,
                     lambda jc=jc: v_piece(jc, 0), ("v", jc, 0))
                )
            # V half-1 (heads 2,3): needed from iter ~64; weave 24..60.
            for jc in range(16):
                weaves.append(
                    (62 + 2 * (jc // 2), 24 + jc, 1024,
                     lambda jc=jc: v_piece(jc, 1), ("v", jc, 1))
                )
            # wo: quarter q ready after norm of sweep 8+2q+1 (+transposes).
            for q in range(3):
                s_done = 8 + 2 * q + 1
                rdy = cit[(s_done, 7)] + 4
                for ib4 in range(4):
                    for eh in range(2):
                        weaves.append(
                            (min(rdy + 10, 127), rdy, 1024,
                             lambda q=q, ib4=ib4, eh=eh: wo_piece(q * 4 + ib4, eh))
                        )
            # q3: c0 halves early (ctxT c0 ready after sweep 7 + transposes),
            # c1 halves + add + store in the tail.
            for ib4 in range(4):
                for eh in range(2):
                    weaves.append(
                        (80, cit[(7, 7)] + 4, 512,
                         lambda ib4=ib4, eh=eh: wo_c0_piece(12 + ib4, eh))
                    )
                    weaves.append(
                        (200 + 2 * ib4 + eh, 131, 512,
                         lambda ib4=ib4, eh=eh: wo_c1_piece(12 + ib4, eh))
                    )
            weaves = [w if len(w) == 5 else (*w, None) for w in weaves]
            weaves.sort(key=lambda u: u[0])
            pending = list(weaves)

            def dummies(n):
                """n x 256 dummy rows to keep the PE p-state ramp alive
                while the head waits on input DMAs."""
                dps = wo_ps.tile([P, 512], F32, tag="wo", name="warmmm")
                for _ in range(n):
                    nc.tensor.matmul(
                        dps[:, 0:256], dmy, dmy2, start=True, stop=True
                    )

            with nc.named_scope("head"):
                dmy = dmy_p.tile([P, P], FP16, tag="dmy")
                dmy2 = dmy_p.tile([P, 256], FP16, tag="dmy2")
                nc.vector.memset(dmy, 0.0)
                nc.vector.memset(dmy2, 0.0)
                dummies(13)  # ramp to full clock by ~2.9us
                qk_sub(wk_sb, xk_t, KT, 0, 0)
                dummies(10)  # bridge to xq e0 arrival ~5.8us
                qk_sub(wq_sb, xq_t, QT, 0, 0)
                dummies(3)  # bridge to xq e1 arrival ~7.3us
                qk_sub(wq_sb, xq_t, QT, 0, 1)

            debt = 0
            for t in range(131):
                with nc.named_scope(f"it{t}"):
                    used = 0
                    if t in scores_at:
                        s, i2 = scores_at[t]
                        if i2 == 0:
                            cx_tiles[s] = cx_ps.tile(
                                [P, 4, DK + 1], F32, tag="cx", name=f"cx{s}"
                            )
                            nc.vector.memset(cx_tiles[s], 0.0)
                        scores_exp(s, i2)
                        used += 1024
                    for key in ctx_at.get(t, []):
                        s2k, i22k = key
                        h2k = 2 * (s2k // 8) + (s2k % 2)
                        need = {("v", 2 * i22k, h2k // 2),
                                ("v", 2 * i22k + 1, h2k // 2)}
                        for i in range(len(pending) - 1, -1, -1):
                            if pending[i][4] in need:
                                ent = pending.pop(i)
                                ent[3]()
                                used += ent[2]
                        ctx_mm(*key)
                        used += 520
                        if key in norm_after:
                            norm_sweep(norm_after[key])
                    if t >= 128:
                        used = -(10**9)
                    while pending:
                        idx = None
                        for i, (dl, rdy, cyc, fn, wkey) in enumerate(pending):
                            debt_after = max(0, debt + used + cyc - CAP)
                            if rdy <= t and (debt_after <= MAX_DEBT or dl <= t):
                                idx = i
                                break
                        if idx is None:
                            break
                        dl, rdy, cyc, fn, wkey = pending.pop(idx)
                        fn()
                        used += cyc
                    debt = max(0, debt + used - CAP) if t < 128 else 0
            with nc.named_scope("tail"):
                for ent in pending:
                    ent[3]()
    nc.compile()
    return nc


def get_nc():
    if not _NC_CACHE:
        _NC_CACHE.append(_build_nc())
    return _NC_CACHE[0]


def kernel(query, key, value, mask, Wq, Wk, Wv, Wo, **_run_kwargs):
    query = np.asarray(query, np.float32)
    key = np.asarray(key, np.float32)
    value = np.asarray(value, np.float32)
    Wq = np.asarray(Wq, np.float32)
    Wk = np.asarray(Wk, np.float32)
    Wv = np.asarray(Wv, np.float32)
    Wo = np.asarray(Wo, np.float32)

    nc = get_nc()
    f16 = np.float16
    in_maps = []
    for b in range(2):
        xqTb = np.ascontiguousarray(query[b].T).astype(f16)
        xkTb = np.ascontiguousarray(key[b].T).astype(f16)
        xvTb = np.ascontiguousarray(value[b].T).astype(f16)
        for g in range(4):
            c0 = g * DKV
            in_maps.append(
                {
                    "xqT": xqTb,
                    "xkT": xkTb,
                    "xvT": xvTb,
                    "wq": np.ascontiguousarray(Wq[:, c0 : c0 + DKV]).astype(f16),
                    "wk": np.ascontiguousarray(Wk[:, c0 : c0 + DKV]).astype(f16),
                    "wv": np.ascontiguousarray(Wv[:, c0 : c0 + DKV]).astype(f16),
                    "wo": np.ascontiguousarray(Wo[c0 : c0 + DKV, :]).astype(f16),
                    "idn": np.eye(P, dtype=f16),
                }
            )
    res = run_bass_kernel_spmd(nc, in_maps, core_ids=list(range(8)), **_run_kwargs)
    outs = [r["out"] for r in res.results]
    full = np.stack(
        [
            outs[0] + outs[1] + outs[2] + outs[3],
            outs[4] + outs[5] + outs[6] + outs[7],
        ]
    ).astype(np.float32)
    if _run_kwargs:
        return full, res
    return full


# revision 3
# speedup vs baseline: 1.0007x; 1.0007x over previous
"""Multi-head attention TRN2 kernel, v2.

Full inputs -> 8-core shard (batch x head-group) -> Bass/Tile kernel -> host
gather+reduce.  Problem: B=2, S=2048, D=1024, H=16, Dk=64, fp32, mask=0.

Core c = b*4 + g handles batch b, heads 4g..4g+3.  All intermediates fp16.

Engine plan (cost-model driven):
  PE    projections (full-128 contraction), scoresT (K=64), ctx with the
        attn chunk STATIONARY and V MOVING (65 rows incl. a ones column so
        the softmax denominator rides along), Wo.
  Act   exp only: one [128,1024] activation per iteration (two j-chunks
        side by side), scale=0.125 folded in.  This stream (~134us busy)
        is the bottleneck; everything else hides under it.
  DVE   psum->sbuf copies + reciprocals.
  Pool  softmax normalize + half the psum->sbuf drains.
  DMA   input stream, ctx transposes via the XBAR, output writes.

Sweeps (i-quarter q, head h) are ordered h-pair-outer: heads 0/1 for all
quarters first (sweeps 0..7 = q0h0,q0h1,q1h0,...), then heads 2/3
(sweeps 8..15).  KT/QT chunk-0 thus feeds the first 8 sweeps and chunk-1
streams in later.  Sweeps 0 and 1 are jc-interleaved into one 16-slot
stretch so the fresh-KT demand rate stays under the HBM stream rate.
ctx matmuls run a few iterations behind their exp (explicit CIT map);
V/QT/KT/Wo pieces fill PE slack via a deadline-greedy budgeter.
PSUM: scores ring 2x2 banks, ctx accums 2x1 bank, wo/proj ring 2x1 bank.
"""

import sys

import numpy as np

try:
    import concourse.bass as bass  # noqa: F401
except ImportError:  # harness runs from a bare directory
    sys.path.insert(0, "/opt/trn_rl_repo")
    import concourse.bass as bass  # noqa: F401

import concourse.tile as tile
from concourse import bacc, mybir
from concourse.bass_utils import run_bass_kernel_spmd

S = 2048
D = 1024
HG = 4  # heads per core
DK = 64
DKV = HG * DK  # 256
P = 128
F32 = mybir.dt.float32
FP16 = mybir.dt.float16
EXP = mybir.ActivationFunctionType.Exp

_NC_CACHE = []

AT_RING = 15
CAP = 2400  # PE cycles per Act period less per-instr overheads
MAX_DEBT = 1200  # PE-behind allowance absorbed by the sc ring


def _iteration_maps():
    """SIT: (s, i2) -> scores iteration; CIT: ctx iteration; both 0-based
    over 128 slots.  Sweeps 0/1 are interleaved over slots 0..15."""
    sit = {}
    for s in range(16):
        for i2 in range(8):
            if s < 2:
                sit[(s, i2)] = 2 * i2 + s
            else:
                sit[(s, i2)] = 8 * s + i2

    lag = {0: 12, 1: 12, 2: 12, 3: 11, 4: 10, 5: 9, 6: 8, 7: 7,
           8: 6, 9: 5, 10: 4, 11: 3, 12: 2, 13: 2, 14: 2, 15: 1}
    cit = {}
    for s in range(16):
        L = lag.get(s, 2)
        for i2 in range(8):
            cit[(s, i2)] = sit[(s, i2)] + L
    return sit, cit


def _build_nc():
    nc = bacc.Bacc("TRN2", target_bir_lowering=False, debug=False)
    xqT = nc.dram_tensor("xqT", [D, S], FP16, kind="ExternalInput")
    xkT = nc.dram_tensor("xkT", [D, S], FP16, kind="ExternalInput")
    xvT = nc.dram_tensor("xvT", [D, S], FP16, kind="ExternalInput")
    wq = nc.dram_tensor("wq", [D, DKV], FP16, kind="ExternalInput")
    wk = nc.dram_tensor("wk", [D, DKV], FP16, kind="ExternalInput")
    wv = nc.dram_tensor("wv", [D, DKV], FP16, kind="ExternalInput")
    wo = nc.dram_tensor("wo", [DKV, D], FP16, kind="ExternalInput")
    idn = nc.dram_tensor("idn", [P, P], FP16, kind="ExternalInput")
    out = nc.dram_tensor("out", [S, D], F32, kind="ExternalOutput")

    sit, cit = _iteration_maps()
    scores_at = {}  # iter -> (s, i2)
    for k, t in sit.items():
        scores_at[t] = k
    ctx_at = {}
    for (s, i2), t in cit.items():
        ctx_at.setdefault(t, []).append((s, i2))
        ctx_at[t].sort(key=lambda k: (cit[k], k))
    norm_after = {}  # (s, i2) -> s to normalize right after that ctx
    for s in range(16):
        norm_after[(s, 7)] = s

    with tile.TileContext(nc) as tc:
        with (
            tc.tile_pool(name="persist", bufs=1) as persist,
            tc.tile_pool(name="xk_p", bufs=8) as xk_p,
            tc.tile_pool(name="xq_p", bufs=8) as xq_p,
            tc.tile_pool(name="xv_p", bufs=4) as xv_p,
            tc.tile_pool(name="at_p", bufs=AT_RING) as at_p,
            tc.tile_pool(name="stage", bufs=2) as stage_p,
            tc.tile_pool(name="q3st", bufs=4) as q3st_p,
            tc.tile_pool(name="small", bufs=4) as small_p,
            tc.tile_pool(name="dmy", bufs=1) as dmy_p,
            tc.tile_pool(name="sc_ps", bufs=2, space="PSUM") as sc_ps,
            tc.tile_pool(name="cx_ps", bufs=2, space="PSUM") as cx_ps,
            tc.tile_pool(name="wo_ps", bufs=2, space="PSUM") as wo_ps,
        ):
            # ---- persistent SBUF ----
            QT = persist.tile([P, 2, S], FP16)  # [(h%2)*64+dk, h//2, i]
            KT = persist.tile([P, 2, S], FP16)
            Vag = persist.tile([P, HG, 16, DK + 1], FP16)  # [j%128, h, jc, dk|1]
            ctx_sb = persist.tile([P, 16, DKV], FP16)  # [i%128, ib, dkv]
            ctxT = persist.tile([P, 2, S], FP16)  # [dkv%128, dkv//128, i]
            wq_sb = persist.tile([P, 8, DKV], FP16)
            wk_sb = persist.tile([P, 8, DKV], FP16)
            wv_sb = persist.tile([P, 8, DKV], FP16)
            wo_sb = persist.tile([P, 2, D], FP16)
            idn_sb = persist.tile([P, P], FP16)

            # ---- head: exp-table warm + ones column ----
            warm = small_p.tile([P, 8], F32, tag="warm")
            nc.vector.memset(warm[0:1, :], 0.0)
            nc.scalar.activation(warm[0:1, :], warm[0:1, :], EXP, scale=0.0)
            nc.vector.memset(Vag[:, :, :, DK : DK + 1], 1.0)

            xk_t, xq_t, xv_t = {}, {}, {}

            def load_x(pool, store, xT, idx, width, name):
                t = pool.tile([P, 8, width], FP16, tag="x", name=f"{name}{idx}")
                nc.sync.dma_start(
                    t,
                    xT.rearrange("(ko p) i -> p ko i", p=P)[
                        :, :, idx * width : (idx + 1) * width
                    ],
                )
                store[idx] = t

            # DMA issue order = arrival priority (in-order SP queue).
            nc.sync.dma_start(wk_sb, wk.rearrange("(ko p) n -> p ko n", p=P))
            load_x(xk_p, xk_t, xkT, 0, 256, "xk")
            nc.sync.dma_start(wq_sb, wq.rearrange("(ko p) n -> p ko n", p=P))
            load_x(xq_p, xq_t, xqT, 0, 256, "xq")
            load_x(xq_p, xq_t, xqT, 1, 256, "xq")
            for e in range(1, 8):
                load_x(xk_p, xk_t, xkT, e, 256, "xk")
            nc.sync.dma_start(wv_sb, wv.rearrange("(ko p) n -> p ko n", p=P))
            load_x(xv_p, xv_t, xvT, 0, 512, "xv")
            load_x(xq_p, xq_t, xqT, 2, 256, "xq")
            load_x(xq_p, xq_t, xqT, 3, 256, "xq")
            load_x(xv_p, xv_t, xvT, 1, 512, "xv")
            load_x(xv_p, xv_t, xvT, 2, 512, "xv")
            load_x(xv_p, xv_t, xvT, 3, 512, "xv")
            load_x(xq_p, xq_t, xqT, 4, 256, "xq")
            load_x(xq_p, xq_t, xqT, 5, 256, "xq")
            load_x(xq_p, xq_t, xqT, 6, 256, "xq")
            load_x(xq_p, xq_t, xqT, 7, 256, "xq")
            nc.sync.dma_start(wo_sb, wo.rearrange("(c p) n -> p c n", p=P))
            nc.sync.dma_start(idn_sb, idn[:, :])

            # ---- piece emitters (all 256-wide sub-pieces for KT/QT) ----
            qk_open = {}

            def qk_sub(w_sb, x_t, OUT, c, e, part=2):
                """OUT[:, c, e*256:(e+1)*256] from x eighth e (2048 cycles);
                part=0/1 emit one 1024-cycle half of the k-accumulation."""
                key = (id(w_sb), c, e)
                if part in (0, 2):
                    qk_open[key] = wo_ps.tile([P, 512], F32, tag="wo", name="qk")
                ps = qk_open[key]
                ks = range(8) if part == 2 else range(4 * part, 4 * part + 4)
                for k in ks:
                    nc.tensor.matmul(
                        ps[:, 0:256],
                        w_sb[:, k, c * P : (c + 1) * P],
                        x_t[e][:, k, :],
                        start=(k == 0),
                        stop=(k == 7),
                    )
                if part in (1, 2):
                    nc.vector.tensor_copy(
                        OUT[:, c, e * 256 : (e + 1) * 256], ps[:, 0:256]
                    )
                    del qk_open[key]

            def v_piece(jc, half):
                """Vag[:, 2*half:2*half+2, jc, 0:64] (1024 cycles)."""
                ps = wo_ps.tile([P, 512], F32, tag="wo", name="v")
                xt = xv_t[jc // 4]
                j0 = (jc % 4) * P
                for k in range(8):
                    nc.tensor.matmul(
                        ps[:, 0:P],
                        xt[:, k, j0 : j0 + P],
                        wv_sb[:, k, half * P : (half + 1) * P],
                        start=(k == 0),
                        stop=(k == 7),
                    )
                nc.vector.tensor_copy(
                    Vag[:, 2 * half : 2 * half + 2, jc, 0:DK],
                    ps[:, 0:P].rearrange("p (h d) -> p h d", h=2),
                )

            stages = {}
            wo_done = {}

            wo_open = {}

            def wo_piece(ib, eh, part=2):
                if ib not in stages:
                    stages[ib] = stage_p.tile([P, D], F32, tag="st", name=f"st{ib}")
                if part in (0, 2):
                    wo_open[(ib, eh)] = wo_ps.tile([P, 512], F32, tag="wo",
                                                   name="woo")
                ps = wo_open[(ib, eh)]
                cs = range(2) if part == 2 else [part]
                for c in cs:
                    nc.tensor.matmul(
                        ps,
                        ctxT[:, c, ib * P : (ib + 1) * P],
                        wo_sb[:, c, eh * 512 : (eh + 1) * 512],
                        start=(c == 0),
                        stop=(c == 1),
                    )
                if part in (1, 2):
                    del wo_open[(ib, eh)]
                    st = stages[ib]
                    nc.vector.tensor_copy(st[:, eh * 512 : (eh + 1) * 512], ps)
                    done = wo_done.setdefault(ib, set())
                    done.add(eh)
                    if done == {0, 1}:
                        nc.sync.dma_start(out[ib * P : (ib + 1) * P, :], st)

            # Last quarter: wo split into an early c0 half (staged to SBUF)
            # and a small tail c1 half + add, to shorten the tail chain.
            def wo_c0_piece(ib, eh):
                if ib not in stages:
                    stages[ib] = q3st_p.tile([P, D], F32, tag="q3st",
                                             name=f"q3st{ib}")
                ps = wo_ps.tile([P, 512], F32, tag="wo", name="woa")
                nc.tensor.matmul(
                    ps,
                    ctxT[:, 0, ib * P : (ib + 1) * P],
                    wo_sb[:, 0, eh * 512 : (eh + 1) * 512],
                    start=True,
                    stop=True,
                )
                eng = nc.vector
                eng.tensor_copy(stages[ib][:, eh * 512 : (eh + 1) * 512], ps)

            wo_c1_ps = {}

            def wo_c1_piece(ib, eh):
                if ib not in wo_c1_ps:
                    wo_c1_ps[ib] = sc_ps.tile([P, 1024], F32, tag="sc", name="wob")
                ps = wo_c1_ps[ib][:, eh * 512 : (eh + 1) * 512]
                nc.tensor.matmul(
                    ps,
                    ctxT[:, 1, ib * P : (ib + 1) * P],
                    wo_sb[:, 1, eh * 512 : (eh + 1) * 512],
                    start=True,
                    stop=True,
                )
                st = stages[ib]
                sl = st[:, eh * 512 : (eh + 1) * 512]
                eng = nc.vector
                eng.scalar_tensor_tensor(
                    sl, ps, 1.0, sl, mybir.AluOpType.mult, mybir.AluOpType.add
                )
                nc.sync.dma_start(
                    out[ib * P : (ib + 1) * P, eh * 512 : (eh + 1) * 512], sl
                )

            cx_tiles = {}
            at_tiles = {}

            def scores_exp(s, i2):
                q, h = (s % 8) // 2, 2 * (s // 8) + (s % 2)
                c, r0 = h // 2, (h % 2) * DK
                sc = sc_ps.tile([P, 1024], F32, tag="sc", name="sc")
                for jj in range(2):
                    jc = 2 * i2 + jj
                    nc.tensor.matmul(
                        sc[:, jj * 512 : (jj + 1) * 512],
                        KT[r0 : r0 + DK, c, jc * P : (jc + 1) * P],
                        QT[r0 : r0 + DK, c, q * 512 : (q + 1) * 512],
                        start=True,
                        stop=True,
                    )
                at = at_p.tile([P, 1024], FP16, tag="at", name=f"at{s}_{i2}")
                nc.scalar.activation(at, sc, EXP, scale=0.125)
                at_tiles[(s, i2)] = at

            def ctx_mm(s, i2):
                q, h = (s % 8) // 2, 2 * (s // 8) + (s % 2)
                at = at_tiles.pop((s, i2))
                cx = cx_tiles[s]
                for jj in range(2):
                    jc = 2 * i2 + jj
                    for ib in range(4):
                        nc.tensor.matmul(
                            cx[:, ib, :],
                            at[:, jj * 512 + ib * P : jj * 512 + (ib + 1) * P],
                            Vag[:, h, jc, :],
                            start=False,
                            stop=(i2 == 7 and jj == 1),
                            skip_group_check=True,
                        )

            def norm_sweep(s):
                q, h = (s % 8) // 2, 2 * (s // 8) + (s % 2)
                cx = cx_tiles.pop(s)
                second = h % 2 == 1  # both heads of chunk h//2 done
                rc = small_p.tile([P, 4], F32, tag="rc")
                with nc.allow_low_precision("softmax denom"):
                    nc.vector.reciprocal(
                        rc, cx[:, :, DK : DK + 1].rearrange("p a b -> p (a b)")
                    )
                for ib in range(4):
                    nc.vector.tensor_scalar_mul(
                        ctx_sb[:, q * 4 + ib, h * DK : (h + 1) * DK],
                        cx[:, ib, 0:DK],
                        rc[:, ib : ib + 1],
                    )
                    if second and s != 15:
                        nc.sync.dma_start_transpose(
                            ctxT[:, h // 2, (q * 4 + ib) * P : (q * 4 + ib + 1) * P],
                            ctx_sb[:, q * 4 + ib, (h // 2) * P : (h // 2 + 1) * P],
                        )
                if s == 15:  # tail: PE transposes (no HWDGE serialization)
                    for ib in range(4):
                        tp = wo_ps.tile([P, P], FP16, tag="wo", name="tp")
                        nc.tensor.transpose(
                            tp, ctx_sb[:, 12 + ib, P : 2 * P], idn_sb
                        )
                        eng = nc.vector
                        eng.tensor_copy(
                            ctxT[:, 1, (12 + ib) * P : (13 + ib) * P], tp
                        )

            # ---- weave units: (deadline, ready, cycles, fn) ----
            weaves = []
            # KT c0 eighths 1..7 (e0 in head): deadline iter 2e (pair-0).
            kt0_rdy = [0, 1, 2, 3, 5, 6, 8, 9]
            for e in range(1, 8):
                for part in range(2):
                    weaves.append(
                        (2 * e - 1, kt0_rdy[e], 1024,
                         lambda e=e, part=part: qk_sub(wk_sb, xk_t, KT, 0, e, part))
                    )
            # KT c1 eighths: needed from iter 64; weave in 20..60.
            for e in range(8):
                for part in range(2):
                    weaves.append(
                        (40 + 2 * e, 14 + e, 1024,
                         lambda e=e, part=part: qk_sub(wk_sb, xk_t, KT, 1, e, part))
                    )
            # QT c0 eighths 2..7 (e0,e1 in head): (c0, qi) by iter 16*qi.
            for e in range(2, 8):
                qi = e // 2
                for part in range(2):
                    weaves.append(
                        (16 * qi - 1, {2: 13, 3: 14, 4: 25, 5: 26, 6: 27, 7: 29}[e],
                         1024, lambda e=e, part=part: qk_sub(wq_sb, xq_t, QT, 0, e, part))
                    )
            # QT c1 eighths: by iter 64 + 16*qi; weave 22..60.
            for e in range(8):
                qi = e // 2
                for part in range(2):
                    weaves.append(
                        (63 + 16 * qi, 22 + e if e < 4 else 25 + e, 1024,
                         lambda e=e, part=part: qk_sub(wq_sb, xq_t, QT, 1, e, part))
                    )
            # V half-0 (heads 0,1): V[jc] by ctx of pair-0 at iter 2*(jc//2)+12.
            for jc in range(16):
                rdy = [11, 14, 16, 17][jc // 4]
                weaves.append(
                    (max(2 * (jc // 2) + 11, rdy), rdy, 1024,
                     lambda jc=jc: v_piece(jc, 0), ("v", jc, 0))
                )
            # V half-1 (heads 2,3): needed from iter ~64; weave 24..60.
            for jc in range(16):
                weaves.append(
                    (62 + 2 * (jc // 2), 18 + jc // 2, 1024,
                     lambda jc=jc: v_piece(jc, 1), ("v", jc, 1))
                )
            # wo: quarter q ready after norm of sweep 8+2q+1 (+transposes).
            for q in range(3):
                s_done = 8 + 2 * q + 1
                rdy = cit[(s_done, 7)] + 4
                for ib4 in range(4):
                    for eh in range(2):
                        for part in range(2):
                            weaves.append(
                                (min(rdy + 10, 127), rdy, 512,
                                 lambda q=q, ib4=ib4, eh=eh, part=part:
                                     wo_piece(q * 4 + ib4, eh, part))
                            )
            # q3: c0 halves early (ctxT c0 ready after sweep 7 + transposes),
            # c1 halves + add + store in the tail.
            for ib4 in range(4):
                for eh in range(2):
                    weaves.append(
                        (80, cit[(7, 7)] + 4, 512,
                         lambda ib4=ib4, eh=eh: wo_c0_piece(12 + ib4, eh))
                    )
                    weaves.append(
                        (200 + 2 * ib4 + eh, 131, 512,
                         lambda ib4=ib4, eh=eh: wo_c1_piece(12 + ib4, eh))
                    )
            weaves = [w if len(w) == 5 else (*w, None) for w in weaves]
            weaves.sort(key=lambda u: u[0])
            pending = list(weaves)

            def dummies(n):
                """n x 256 dummy rows to keep the PE p-state ramp alive
                while the head waits on input DMAs."""
                dps = wo_ps.tile([P, 512], F32, tag="wo", name="warmmm")
                for _ in range(n):
                    nc.tensor.matmul(
                        dps[:, 0:256], dmy, dmy2, start=True, stop=True
                    )

            with nc.named_scope("head"):
                dmy = dmy_p.tile([P, P], FP16, tag="dmy")
                dmy2 = dmy_p.tile([P, 256], FP16, tag="dmy2")
                nc.vector.memset(dmy, 0.0)
                nc.vector.memset(dmy2, 0.0)
                dummies(13)  # ramp to full clock by ~2.9us
                qk_sub(wk_sb, xk_t, KT, 0, 0)
                dummies(10)  # bridge to xq e0 arrival ~5.8us
                qk_sub(wq_sb, xq_t, QT, 0, 0)
                dummies(3)  # bridge to xq e1 arrival ~7.3us
                qk_sub(wq_sb, xq_t, QT, 0, 1)

            debt = 0
            for t in range(131):
                with nc.named_scope(f"it{t}"):
                    used = 0
                    if t in scores_at:
                        s, i2 = scores_at[t]
                        if i2 == 0:
                            cx_tiles[s] = cx_ps.tile(
                                [P, 4, DK + 1], F32, tag="cx", name=f"cx{s}"
                            )
                            nc.vector.memset(cx_tiles[s], 0.0)
                        scores_exp(s, i2)
                        used += 1024
                    for key in ctx_at.get(t, []):
                        s2k, i22k = key
                        h2k = 2 * (s2k // 8) + (s2k % 2)
                        need = {("v", 2 * i22k, h2k // 2),
                                ("v", 2 * i22k + 1, h2k // 2)}
                        for i in range(len(pending) - 1, -1, -1):
                            if pending[i][4] in need:
                                ent = pending.pop(i)
                                ent[3]()
                                used += ent[2]
                        ctx_mm(*key)
                        used += 520
                        if key in norm_after:
                            norm_sweep(norm_after[key])
                    if t >= 128:
                        used = -(10**9)
                    while pending:
                        idx = None
                        for i, (dl, rdy, cyc, fn, wkey) in enumerate(pending):
                            debt_after = max(0, debt + used + cyc - CAP)
                            if rdy <= t and (debt_after <= MAX_DEBT or dl <= t):
                                idx = i
                                break
                        if idx is None:
                            break
                        dl, rdy, cyc, fn, wkey = pending.pop(idx)
                        fn()
                        used += cyc
                    debt = max(0, debt + used - CAP) if t < 128 else 0
            with nc.named_scope("tail"):
                for ent in pending:
                    ent[3]()
    nc.compile()
    return nc


def get_nc():
    if not _NC_CACHE:
        _NC_CACHE.append(_build_nc())
    return _NC_CACHE[0]


def kernel(query, key, value, mask, Wq, Wk, Wv, Wo, **_run_kwargs):
    query = np.asarray(query, np.float32)
    key = np.asarray(key, np.float32)
    value = np.asarray(value, np.float32)
    Wq = np.asarray(Wq, np.float32)
    Wk = np.asarray(Wk, np.float32)
    Wv = np.asarray(Wv, np.float32)
    Wo = np.asarray(Wo, np.float32)

    nc = get_nc()
    f16 = np.float16
    in_maps = []
    for b in range(2):
        xqTb = np.ascontiguousarray(query[b].T).astype(f16)
        xkTb = np.ascontiguousarray(key[b].T).astype(f16)
        xvTb = np.ascontiguousarray(value[b].T).astype(f16)
        for g in range(4):
            c0 = g * DKV
            in_maps.append(
                {
                    "xqT": xqTb,
                    "xkT": xkTb,
                    "xvT": xvTb,
                    "wq": np.ascontiguousarray(Wq[:, c0 : c0 + DKV]).astype(f16),
                    "wk": np.ascontiguousarray(Wk[:, c0 : c0 + DKV]).astype(f16),
                    "wv": np.ascontiguousarray(Wv[:, c0 : c0 + DKV]).astype(f16),
                    "wo": np.ascontiguousarray(Wo[c0 : c0 + DKV, :]).astype(f16),
                    "idn": np.eye(P, dtype=f16),
                }
            )
    res = run_bass_kernel_spmd(nc, in_maps, core_ids=list(range(8)), **_run_kwargs)
    outs = [r["out"] for r in res.results]
    full = np.stack(
        [
            outs[0] + outs[1] + outs[2] + outs[3],
            outs[4] + outs[5] + outs[6] + outs[7],
        ]
    ).astype(np.float32)
    if _run_kwargs:
        return full, res
    return full


# revision 4
# speedup vs baseline: 1.0047x; 1.0040x over previous
"""Multi-head attention TRN2 kernel, v2.

Full inputs -> 8-core shard (batch x head-group) -> Bass/Tile kernel -> host
gather+reduce.  Problem: B=2, S=2048, D=1024, H=16, Dk=64, fp32, mask=0.

Core c = b*4 + g handles batch b, heads 4g..4g+3.  All intermediates fp16.

Engine plan (cost-model driven):
  PE    projections (full-128 contraction), scoresT (K=64), ctx with the
        attn chunk STATIONARY and V MOVING (65 rows incl. a ones column so
        the softmax denominator rides along), Wo.
  Act   exp only: one [128,1024] activation per iteration (two j-chunks
        side by side), scale=0.125 folded in.  This stream (~134us busy)
        is the bottleneck; everything else hides under it.
  DVE   psum->sbuf copies + reciprocals.
  Pool  softmax normalize + half the psum->sbuf drains.
  DMA   input stream, ctx transposes via the XBAR, output writes.

Sweeps (i-quarter q, head h) are ordered h-pair-outer: heads 0/1 for all
quarters first (sweeps 0..7 = q0h0,q0h1,q1h0,...), then heads 2/3
(sweeps 8..15).  KT/QT chunk-0 thus feeds the first 8 sweeps and chunk-1
streams in later.  Sweeps 0 and 1 are jc-interleaved into one 16-slot
stretch so the fresh-KT demand rate stays under the HBM stream rate.
ctx matmuls run a few iterations behind their exp (explicit CIT map);
V/QT/KT/Wo pieces fill PE slack via a deadline-greedy budgeter.
PSUM: scores ring 2x2 banks, ctx accums 2x1 bank, wo/proj ring 2x1 bank.
"""

import sys

import numpy as np

try:
    import concourse.bass as bass  # noqa: F401
except ImportError:  # harness runs from a bare directory
    sys.path.insert(0, "/opt/trn_rl_repo")
    import concourse.bass as bass  # noqa: F401

import concourse.tile as tile
from concourse import bacc, mybir
from concourse.bass_utils import run_bass_kernel_spmd

S = 2048
D = 1024
HG = 4  # heads per core
DK = 64
DKV = HG * DK  # 256
P = 128
F32 = mybir.dt.float32
FP16 = mybir.dt.float16
EXP = mybir.ActivationFunctionType.Exp

_NC_CACHE = []

AT_RING = 15
CAP = 2400  # PE cycles per Act period less per-instr overheads
MAX_DEBT = 1200  # PE-behind allowance absorbed by the sc ring


def _iteration_maps():
    """SIT: (s, i2) -> scores iteration; CIT: ctx iteration; both 0-based
    over 128 slots.  Sweeps 0/1 are interleaved over slots 0..15."""
    sit = {}
    for s in range(16):
        for i2 in range(8):
            if s < 2:
                sit[(s, i2)] = 2 * i2 + s
            else:
                sit[(s, i2)] = 8 * s + i2

    lag = {0: 11, 1: 11, 2: 11, 3: 10, 4: 9, 5: 8, 6: 7, 7: 6,
           8: 5, 9: 4, 10: 3, 11: 2, 12: 2, 13: 2, 14: 2, 15: 1}
    cit = {}
    for s in range(16):
        L = lag.get(s, 2)
        for i2 in range(8):
            cit[(s, i2)] = sit[(s, i2)] + L
    return sit, cit


def _build_nc():
    nc = bacc.Bacc("TRN2", target_bir_lowering=False, debug=False)
    xqT = nc.dram_tensor("xqT", [D, S], FP16, kind="ExternalInput")
    xkT = nc.dram_tensor("xkT", [D, S], FP16, kind="ExternalInput")
    xvT = nc.dram_tensor("xvT", [D, S], FP16, kind="ExternalInput")
    wq = nc.dram_tensor("wq", [D, DKV], FP16, kind="ExternalInput")
    wk = nc.dram_tensor("wk", [D, DKV], FP16, kind="ExternalInput")
    wv = nc.dram_tensor("wv", [D, DKV], FP16, kind="ExternalInput")
    wo = nc.dram_tensor("wo", [DKV, D], FP16, kind="ExternalInput")
    idn = nc.dram_tensor("idn", [P, P], FP16, kind="ExternalInput")
    out = nc.dram_tensor("out", [S, D], F32, kind="ExternalOutput")

    sit, cit = _iteration_maps()
    scores_at = {}  # iter -> (s, i2)
    for k, t in sit.items():
        scores_at[t] = k
    ctx_at = {}
    for (s, i2), t in cit.items():
        ctx_at.setdefault(t, []).append((s, i2))
        ctx_at[t].sort(key=lambda k: (cit[k], k))
    norm_after = {}  # (s, i2) -> s to normalize right after that ctx
    for s in range(16):
        norm_after[(s, 7)] = s

    with tile.TileContext(nc) as tc:
        with (
            tc.tile_pool(name="persist", bufs=1) as persist,
            tc.tile_pool(name="xk_p", bufs=8) as xk_p,
            tc.tile_pool(name="xq_p", bufs=8) as xq_p,
            tc.tile_pool(name="xv_p", bufs=4) as xv_p,
            tc.tile_pool(name="at_p", bufs=AT_RING) as at_p,
            tc.tile_pool(name="stage", bufs=2) as stage_p,
            tc.tile_pool(name="q3st", bufs=4) as q3st_p,
            tc.tile_pool(name="small", bufs=4) as small_p,
            tc.tile_pool(name="dmy", bufs=1) as dmy_p,
            tc.tile_pool(name="sc_ps", bufs=2, space="PSUM") as sc_ps,
            tc.tile_pool(name="cx_ps", bufs=2, space="PSUM") as cx_ps,
            tc.tile_pool(name="wo_ps", bufs=2, space="PSUM") as wo_ps,
        ):
            # ---- persistent SBUF ----
            QT = persist.tile([P, 2, S], FP16)  # [(h%2)*64+dk, h//2, i]
            KT = persist.tile([P, 2, S], FP16)
            Vag = persist.tile([P, HG, 16, DK + 1], FP16)  # [j%128, h, jc, dk|1]
            ctx_sb = persist.tile([P, 16, DKV], FP16)  # [i%128, ib, dkv]
            ctxT = persist.tile([P, 2, S], FP16)  # [dkv%128, dkv//128, i]
            wq_sb = persist.tile([P, 8, DKV], FP16)
            wk_sb = persist.tile([P, 8, DKV], FP16)
            wv_sb = persist.tile([P, 8, DKV], FP16)
            wo_sb = persist.tile([P, 2, D], FP16)
            idn_sb = persist.tile([P, P], FP16)

            # ---- head: exp-table warm + ones column ----
            warm = small_p.tile([P, 8], F32, tag="warm")
            nc.vector.memset(warm[0:1, :], 0.0)
            nc.scalar.activation(warm[0:1, :], warm[0:1, :], EXP, scale=0.0)
            nc.vector.memset(Vag[:, :, :, DK : DK + 1], 1.0)

            xk_t, xq_t, xv_t = {}, {}, {}

            def load_x(pool, store, xT, idx, width, name):
                t = pool.tile([P, 8, width], FP16, tag="x", name=f"{name}{idx}")
                nc.sync.dma_start(
                    t,
                    xT.rearrange("(ko p) i -> p ko i", p=P)[
                        :, :, idx * width : (idx + 1) * width
                    ],
                )
                store[idx] = t

            # DMA issue order = arrival priority (in-order SP queue).
            nc.sync.dma_start(wk_sb, wk.rearrange("(ko p) n -> p ko n", p=P))
            load_x(xk_p, xk_t, xkT, 0, 256, "xk")
            nc.sync.dma_start(wq_sb, wq.rearrange("(ko p) n -> p ko n", p=P))
            load_x(xq_p, xq_t, xqT, 0, 256, "xq")
            load_x(xq_p, xq_t, xqT, 1, 256, "xq")
            for e in range(1, 8):
                load_x(xk_p, xk_t, xkT, e, 256, "xk")
            nc.sync.dma_start(wv_sb, wv.rearrange("(ko p) n -> p ko n", p=P))
            load_x(xv_p, xv_t, xvT, 0, 512, "xv")
            load_x(xq_p, xq_t, xqT, 2, 256, "xq")
            load_x(xq_p, xq_t, xqT, 3, 256, "xq")
            load_x(xv_p, xv_t, xvT, 1, 512, "xv")
            load_x(xv_p, xv_t, xvT, 2, 512, "xv")
            load_x(xv_p, xv_t, xvT, 3, 512, "xv")
            load_x(xq_p, xq_t, xqT, 4, 256, "xq")
            load_x(xq_p, xq_t, xqT, 5, 256, "xq")
            load_x(xq_p, xq_t, xqT, 6, 256, "xq")
            load_x(xq_p, xq_t, xqT, 7, 256, "xq")
            nc.sync.dma_start(wo_sb, wo.rearrange("(c p) n -> p c n", p=P))
            nc.sync.dma_start(idn_sb, idn[:, :])

            # ---- piece emitters (all 256-wide sub-pieces for KT/QT) ----
            qk_open = {}

            def qk_sub(w_sb, x_t, OUT, c, e, part=2):
                """OUT[:, c, e*256:(e+1)*256] from x eighth e (2048 cycles);
                part=0/1 emit one 1024-cycle half of the k-accumulation."""
                key = (id(w_sb), c, e)
                if part in (0, 2):
                    qk_open[key] = wo_ps.tile([P, 512], F32, tag="wo", name="qk")
                ps = qk_open[key]
                ks = range(8) if part == 2 else range(4 * part, 4 * part + 4)
                for k in ks:
                    nc.tensor.matmul(
                        ps[:, 0:256],
                        w_sb[:, k, c * P : (c + 1) * P],
                        x_t[e][:, k, :],
                        start=(k == 0),
                        stop=(k == 7),
                    )
                if part in (1, 2):
                    nc.vector.tensor_copy(
                        OUT[:, c, e * 256 : (e + 1) * 256], ps[:, 0:256]
                    )
                    del qk_open[key]

            def v_piece(jc, half):
                """Vag[:, 2*half:2*half+2, jc, 0:64] (1024 cycles)."""
                ps = wo_ps.tile([P, 512], F32, tag="wo", name="v")
                xt = xv_t[jc // 4]
                j0 = (jc % 4) * P
                for k in range(8):
                    nc.tensor.matmul(
                        ps[:, 0:P],
                        xt[:, k, j0 : j0 + P],
                        wv_sb[:, k, half * P : (half + 1) * P],
                        start=(k == 0),
                        stop=(k == 7),
                    )
                nc.vector.tensor_copy(
                    Vag[:, 2 * half : 2 * half + 2, jc, 0:DK],
                    ps[:, 0:P].rearrange("p (h d) -> p h d", h=2),
                )

            stages = {}
            wo_done = {}

            wo_open = {}

            def wo_piece(ib, eh, part=2):
                if ib not in stages:
                    stages[ib] = stage_p.tile([P, D], F32, tag="st", name=f"st{ib}")
                if part in (0, 2):
                    wo_open[(ib, eh)] = wo_ps.tile([P, 512], F32, tag="wo",
                                                   name="woo")
                ps = wo_open[(ib, eh)]
                cs = range(2) if part == 2 else [part]
                for c in cs:
                    nc.tensor.matmul(
                        ps,
                        ctxT[:, c, ib * P : (ib + 1) * P],
                        wo_sb[:, c, eh * 512 : (eh + 1) * 512],
                        start=(c == 0),
                        stop=(c == 1),
                    )
                if part in (1, 2):
                    del wo_open[(ib, eh)]
                    st = stages[ib]
                    nc.vector.tensor_copy(st[:, eh * 512 : (eh + 1) * 512], ps)
                    done = wo_done.setdefault(ib, set())
                    done.add(eh)
                    if done == {0, 1}:
                        nc.sync.dma_start(out[ib * P : (ib + 1) * P, :], st)

            # Last quarter: wo split into an early c0 half (staged to SBUF)
            # and a small tail c1 half + add, to shorten the tail chain.
            def wo_c0_piece(ib, eh):
                if ib not in stages:
                    stages[ib] = q3st_p.tile([P, D], F32, tag="q3st",
                                             name=f"q3st{ib}")
                ps = wo_ps.tile([P, 512], F32, tag="wo", name="woa")
                nc.tensor.matmul(
                    ps,
                    ctxT[:, 0, ib * P : (ib + 1) * P],
                    wo_sb[:, 0, eh * 512 : (eh + 1) * 512],
                    start=True,
                    stop=True,
                )
                eng = nc.vector
                eng.tensor_copy(stages[ib][:, eh * 512 : (eh + 1) * 512], ps)

            wo_c1_ps = {}

            def wo_c1_piece(ib, eh):
                if ib not in wo_c1_ps:
                    wo_c1_ps[ib] = sc_ps.tile([P, 1024], F32, tag="sc", name="wob")
                ps = wo_c1_ps[ib][:, eh * 512 : (eh + 1) * 512]
                nc.tensor.matmul(
                    ps,
                    ctxT[:, 1, ib * P : (ib + 1) * P],
                    wo_sb[:, 1, eh * 512 : (eh + 1) * 512],
                    start=True,
                    stop=True,
                )
                st = stages[ib]
                sl = st[:, eh * 512 : (eh + 1) * 512]
                eng = nc.vector
                eng.scalar_tensor_tensor(
                    sl, ps, 1.0, sl, mybir.AluOpType.mult, mybir.AluOpType.add
                )
                nc.sync.dma_start(
                    out[ib * P : (ib + 1) * P, eh * 512 : (eh + 1) * 512], sl
                )

            cx_tiles = {}
            at_tiles = {}

            def scores_exp(s, i2):
                q, h = (s % 8) // 2, 2 * (s // 8) + (s % 2)
                c, r0 = h // 2, (h % 2) * DK
                sc = sc_ps.tile([P, 1024], F32, tag="sc", name="sc")
                for jj in range(2):
                    jc = 2 * i2 + jj
                    nc.tensor.matmul(
                        sc[:, jj * 512 : (jj + 1) * 512],
                        KT[r0 : r0 + DK, c, jc * P : (jc + 1) * P],
                        QT[r0 : r0 + DK, c, q * 512 : (q + 1) * 512],
                        start=True,
                        stop=True,
                    )
                at = at_p.tile([P, 1024], FP16, tag="at", name=f"at{s}_{i2}")
                nc.scalar.activation(at, sc, EXP, scale=0.125)
                at_tiles[(s, i2)] = at

            def ctx_mm(s, i2):
                q, h = (s % 8) // 2, 2 * (s // 8) + (s % 2)
                at = at_tiles.pop((s, i2))
                cx = cx_tiles[s]
                for jj in range(2):
                    jc = 2 * i2 + jj
                    for ib in range(4):
                        nc.tensor.matmul(
                            cx[:, ib, :],
                            at[:, jj * 512 + ib * P : jj * 512 + (ib + 1) * P],
                            Vag[:, h, jc, :],
                            start=False,
                            stop=(i2 == 7 and jj == 1),
                            skip_group_check=True,
                        )

            def norm_sweep(s):
                q, h = (s % 8) // 2, 2 * (s // 8) + (s % 2)
                cx = cx_tiles.pop(s)
                second = h % 2 == 1  # both heads of chunk h//2 done
                rc = small_p.tile([P, 4], F32, tag="rc")
                with nc.allow_low_precision("softmax denom"):
                    nc.vector.reciprocal(
                        rc, cx[:, :, DK : DK + 1].rearrange("p a b -> p (a b)")
                    )
                for ib in range(4):
                    nc.vector.tensor_scalar_mul(
                        ctx_sb[:, q * 4 + ib, h * DK : (h + 1) * DK],
                        cx[:, ib, 0:DK],
                        rc[:, ib : ib + 1],
                    )
                    if second and s != 15:
                        nc.sync.dma_start_transpose(
                            ctxT[:, h // 2, (q * 4 + ib) * P : (q * 4 + ib + 1) * P],
                            ctx_sb[:, q * 4 + ib, (h // 2) * P : (h // 2 + 1) * P],
                        )
                if s == 15:  # tail: PE transposes (no HWDGE serialization)
                    for ib in range(4):
                        tp = wo_ps.tile([P, P], FP16, tag="wo", name="tp")
                        nc.tensor.transpose(
                            tp, ctx_sb[:, 12 + ib, P : 2 * P], idn_sb
                        )
                        eng = nc.vector
                        eng.tensor_copy(
                            ctxT[:, 1, (12 + ib) * P : (13 + ib) * P], tp
                        )

            # ---- weave units: (deadline, ready, cycles, fn) ----
            weaves = []
            # KT c0 eighths 1..7 (e0 in head): deadline iter 2e (pair-0).
            kt0_rdy = [0, 1, 2, 3, 5, 6, 8, 9]
            for e in range(1, 8):
                for part in range(2):
                    weaves.append(
                        (2 * e - 1, kt0_rdy[e], 1024,
                         lambda e=e, part=part: qk_sub(wk_sb, xk_t, KT, 0, e, part))
                    )
            # KT c1 eighths: needed from iter 64; weave in 20..60.
            for e in range(8):
                for part in range(2):
                    weaves.append(
                        (40 + 2 * e, 14 + e, 1024,
                         lambda e=e, part=part: qk_sub(wk_sb, xk_t, KT, 1, e, part))
                    )
            # QT c0 eighths 2..7 (e0,e1 in head): (c0, qi) by iter 16*qi.
            for e in range(2, 8):
                qi = e // 2
                for part in range(2):
                    weaves.append(
                        (16 * qi - 1, {2: 13, 3: 14, 4: 25, 5: 26, 6: 27, 7: 29}[e],
                         1024, lambda e=e, part=part: qk_sub(wq_sb, xq_t, QT, 0, e, part))
                    )
            # QT c1 eighths: by iter 64 + 16*qi; weave 22..60.
            for e in range(8):
                qi = e // 2
                for part in range(2):
                    weaves.append(
                        (63 + 16 * qi, 22 + e if e < 4 else 25 + e, 1024,
                         lambda e=e, part=part: qk_sub(wq_sb, xq_t, QT, 1, e, part))
                    )
            # V half-0 (heads 0,1): V[jc] by ctx of pair-0 at iter 2*(jc//2)+12.
            for jc in range(16):
                rdy = [11, 14, 16, 17][jc // 4]
                weaves.append(
                    (max(2 * (jc // 2) + 11, rdy), rdy, 1024,
                     lambda jc=jc: v_piece(jc, 0), ("v", jc, 0))
                )
            # V half-1 (heads 2,3): needed from iter ~64; weave 24..60.
            for jc in range(16):
                weaves.append(
                    (62 + 2 * (jc // 2), 18 + jc // 2, 1024,
                     lambda jc=jc: v_piece(jc, 1), ("v", jc, 1))
                )
            # wo: quarter q ready after norm of sweep 8+2q+1 (+transposes).
            for q in range(3):
                s_done = 8 + 2 * q + 1
                rdy = cit[(s_done, 7)] + 4
                for ib4 in range(4):
                    for eh in range(2):
                        for part in range(2):
                            weaves.append(
                                (min(rdy + 10, 127), rdy, 512,
                                 lambda q=q, ib4=ib4, eh=eh, part=part:
                                     wo_piece(q * 4 + ib4, eh, part))
                            )
            # q3: c0 halves early (ctxT c0 ready after sweep 7 + transposes),
            # c1 halves + add + store in the tail.
            for ib4 in range(4):
                for eh in range(2):
                    weaves.append(
                        (80, cit[(7, 7)] + 4, 512,
                         lambda ib4=ib4, eh=eh: wo_c0_piece(12 + ib4, eh))
                    )
                    weaves.append(
                        (200 + 2 * ib4 + eh, 131, 512,
                         lambda ib4=ib4, eh=eh: wo_c1_piece(12 + ib4, eh))
                    )
            weaves = [w if len(w) == 5 else (*w, None) for w in weaves]
            weaves.sort(key=lambda u: u[0])
            pending = list(weaves)

            def dummies(n):
                """n x 256 dummy rows to keep the PE p-state ramp alive
                while the head waits on input DMAs."""
                dps = wo_ps.tile([P, 512], F32, tag="wo", name="warmmm")
                for _ in range(n):
                    nc.tensor.matmul(
                        dps[:, 0:256], dmy, dmy2, start=True, stop=True
                    )

            with nc.named_scope("head"):
                dmy = dmy_p.tile([P, P], FP16, tag="dmy")
                dmy2 = dmy_p.tile([P, 256], FP16, tag="dmy2")
                nc.vector.memset(dmy, 0.0)
                nc.vector.memset(dmy2, 0.0)
                dummies(13)  # ramp to full clock by ~2.9us
                qk_sub(wk_sb, xk_t, KT, 0, 0)
                dummies(10)  # bridge to xq e0 arrival ~5.8us
                qk_sub(wq_sb, xq_t, QT, 0, 0)
                dummies(3)  # bridge to xq e1 arrival ~7.3us
                qk_sub(wq_sb, xq_t, QT, 0, 1)

            debt = 0
            for t in range(131):
                with nc.named_scope(f"it{t}"):
                    used = 0
                    if t in scores_at:
                        s, i2 = scores_at[t]
                        if i2 == 0:
                            cx_tiles[s] = cx_ps.tile(
                                [P, 4, DK + 1], F32, tag="cx", name=f"cx{s}"
                            )
                            nc.vector.memset(cx_tiles[s], 0.0)
                        scores_exp(s, i2)
                        used += 1024
                    for key in ctx_at.get(t, []):
                        s2k, i22k = key
                        h2k = 2 * (s2k // 8) + (s2k % 2)
                        need = {("v", 2 * i22k, h2k // 2),
                                ("v", 2 * i22k + 1, h2k // 2)}
                        for i in range(len(pending) - 1, -1, -1):
                            if pending[i][4] in need:
                                ent = pending.pop(i)
                                ent[3]()
                                used += ent[2]
                        ctx_mm(*key)
                        used += 520
                        if key in norm_after:
                            norm_sweep(norm_after[key])
                    if t >= 128:
                        used = -(10**9)
                    while pending:
                        idx = None
                        for i, (dl, rdy, cyc, fn, wkey) in enumerate(pending):
                            debt_after = max(0, debt + used + cyc - CAP)
                            if rdy <= t and (debt_after <= MAX_DEBT or dl <= t):
                                idx = i
                                break
                        if idx is None:
                            break
                        dl, rdy, cyc, fn, wkey = pending.pop(idx)
                        fn()
                        used += cyc
                    debt = max(0, debt + used - CAP) if t < 128 else 0
            with nc.named_scope("tail"):
                for ent in pending:
                    ent[3]()
    nc.compile()
    return nc


def get_nc():
    if not _NC_CACHE:
        _NC_CACHE.append(_build_nc())
    return _NC_CACHE[0]


def kernel(query, key, value, mask, Wq, Wk, Wv, Wo, **_run_kwargs):
    query = np.asarray(query, np.float32)
    key = np.asarray(key, np.float32)
    value = np.asarray(value, np.float32)
    Wq = np.asarray(Wq, np.float32)
    Wk = np.asarray(Wk, np.float32)
    Wv = np.asarray(Wv, np.float32)
    Wo = np.asarray(Wo, np.float32)

    nc = get_nc()
    f16 = np.float16
    in_maps = []
    for b in range(2):
        xqTb = np.ascontiguousarray(query[b].T).astype(f16)
        xkTb = np.ascontiguousarray(key[b].T).astype(f16)
        xvTb = np.ascontiguousarray(value[b].T).astype(f16)
        for g in range(4):
            c0 = g * DKV
            in_maps.append(
                {
                    "xqT": xqTb,
                    "xkT": xkTb,
                    "xvT": xvTb,
                    "wq": np.ascontiguousarray(Wq[:, c0 : c0 + DKV]).astype(f16),
                    "wk": np.ascontiguousarray(Wk[:, c0 : c0 + DKV]).astype(f16),
                    "wv": np.ascontiguousarray(Wv[:, c0 : c0 + DKV]).astype(f16),
                    "wo": np.ascontiguousarray(Wo[c0 : c0 + DKV, :]).astype(f16),
                    "idn": np.eye(P, dtype=f16),
                }
            )
    res = run_bass_kernel_spmd(nc, in_maps, core_ids=list(range(8)), **_run_kwargs)
    outs = [r["out"] for r in res.results]
    full = np.stack(
        [
            outs[0] + outs[1] + outs[2] + outs[3],
            outs[4] + outs[5] + outs[6] + outs[7],
        ]
    ).astype(np.float32)
    if _run_kwargs:
        return full, res
    return full


# revision 5
# speedup vs baseline: 1.0131x; 1.0084x over previous
"""Multi-head attention TRN2 kernel, v2.

Full inputs -> 8-core shard (batch x head-group) -> Bass/Tile kernel -> host
gather+reduce.  Problem: B=2, S=2048, D=1024, H=16, Dk=64, fp32, mask=0.

Core c = b*4 + g handles batch b, heads 4g..4g+3.  All intermediates fp16.

Engine plan (cost-model driven):
  PE    projections (full-128 contraction), scoresT (K=64), ctx with the
        attn chunk STATIONARY and V MOVING (65 rows incl. a ones column so
        the softmax denominator rides along), Wo.
  Act   exp only: one [128,1024] activation per iteration (two j-chunks
        side by side), scale=0.125 folded in.  This stream (~134us busy)
        is the bottleneck; everything else hides under it.
  DVE   psum->sbuf copies + reciprocals.
  Pool  softmax normalize + half the psum->sbuf drains.
  DMA   input stream, ctx transposes via the XBAR, output writes.

Sweeps (i-quarter q, head h) are ordered h-pair-outer: heads 0/1 for all
quarters first (sweeps 0..7 = q0h0,q0h1,q1h0,...), then heads 2/3
(sweeps 8..15).  KT/QT chunk-0 thus feeds the first 8 sweeps and chunk-1
streams in later.  Sweeps 0 and 1 are jc-interleaved into one 16-slot
stretch so the fresh-KT demand rate stays under the HBM stream rate.
ctx matmuls run a few iterations behind their exp (explicit CIT map);
V/QT/KT/Wo pieces fill PE slack via a deadline-greedy budgeter.
PSUM: scores ring 2x2 banks, ctx accums 2x1 bank, wo/proj ring 2x1 bank.
"""

import sys

import numpy as np

try:
    import concourse.bass as bass  # noqa: F401
except ImportError:  # harness runs from a bare directory
    sys.path.insert(0, "/opt/trn_rl_repo")
    import concourse.bass as bass  # noqa: F401

import concourse.tile as tile
from concourse import bacc, mybir
from concourse.bass_utils import run_bass_kernel_spmd

S = 2048
D = 1024
HG = 4  # heads per core
DK = 64
DKV = HG * DK  # 256
P = 128
F32 = mybir.dt.float32
FP16 = mybir.dt.float16
EXP = mybir.ActivationFunctionType.Exp

_NC_CACHE = []

AT_RING = 15
CAP = 2400  # PE cycles per Act period less per-instr overheads
MAX_DEBT = 1200  # PE-behind allowance absorbed by the sc ring


def _iteration_maps():
    """SIT: (s, i2) -> scores iteration; CIT: ctx iteration; both 0-based
    over 128 slots.  Sweeps 0/1 are interleaved over slots 0..15."""
    sit = {}
    for s in range(16):
        for i2 in range(8):
            if s < 2:
                sit[(s, i2)] = 2 * i2 + s
            else:
                sit[(s, i2)] = 8 * s + i2

    lag = {0: 13, 1: 13, 2: 13, 3: 12, 4: 11, 5: 10, 6: 9, 7: 8,
           8: 7, 9: 6, 10: 5, 11: 4, 12: 3, 13: 2, 14: 2, 15: 1}
    cit = {}
    for s in range(16):
        L = lag.get(s, 2)
        for i2 in range(8):
            cit[(s, i2)] = sit[(s, i2)] + L
    return sit, cit


def _build_nc():
    nc = bacc.Bacc("TRN2", target_bir_lowering=False, debug=False)
    xqT = nc.dram_tensor("xqT", [D, S], FP16, kind="ExternalInput")
    xkT = nc.dram_tensor("xkT", [D, S], FP16, kind="ExternalInput")
    xvT = nc.dram_tensor("xvT", [D, S], FP16, kind="ExternalInput")
    wq = nc.dram_tensor("wq", [D, DKV], FP16, kind="ExternalInput")
    wk = nc.dram_tensor("wk", [D, DKV], FP16, kind="ExternalInput")
    wv = nc.dram_tensor("wv", [D, DKV], FP16, kind="ExternalInput")
    wo = nc.dram_tensor("wo", [DKV, D], FP16, kind="ExternalInput")
    idn = nc.dram_tensor("idn", [P, P], FP16, kind="ExternalInput")
    out = nc.dram_tensor("out", [S, D], F32, kind="ExternalOutput")

    sit, cit = _iteration_maps()
    scores_at = {}  # iter -> (s, i2)
    for k, t in sit.items():
        scores_at[t] = k
    ctx_at = {}
    for (s, i2), t in cit.items():
        ctx_at.setdefault(t, []).append((s, i2))
        ctx_at[t].sort(key=lambda k: (cit[k], k))
    norm_after = {}  # (s, i2) -> s to normalize right after that ctx
    for s in range(16):
        norm_after[(s, 7)] = s

    with tile.TileContext(nc) as tc:
        with (
            tc.tile_pool(name="persist", bufs=1) as persist,
            tc.tile_pool(name="xk_p", bufs=8) as xk_p,
            tc.tile_pool(name="xq_p", bufs=8) as xq_p,
            tc.tile_pool(name="xv_p", bufs=4) as xv_p,
            tc.tile_pool(name="at_p", bufs=AT_RING) as at_p,
            tc.tile_pool(name="stage", bufs=2) as stage_p,
            tc.tile_pool(name="q3st", bufs=4) as q3st_p,
            tc.tile_pool(name="small", bufs=4) as small_p,
            tc.tile_pool(name="dmy", bufs=1) as dmy_p,
            tc.tile_pool(name="sc_ps", bufs=2, space="PSUM") as sc_ps,
            tc.tile_pool(name="cx_ps", bufs=2, space="PSUM") as cx_ps,
            tc.tile_pool(name="wo_ps", bufs=2, space="PSUM") as wo_ps,
        ):
            # ---- persistent SBUF ----
            QT = persist.tile([P, 2, S], FP16)  # [(h%2)*64+dk, h//2, i]
            KT = persist.tile([P, 2, S], FP16)
            Vag = persist.tile([P, HG, 16, DK + 1], FP16)  # [j%128, h, jc, dk|1]
            ctx_sb = persist.tile([P, 16, DKV], FP16)  # [i%128, ib, dkv]
            ctxT = persist.tile([P, 2, S], FP16)  # [dkv%128, dkv//128, i]
            wq_sb = persist.tile([P, 8, DKV], FP16)
            wk_sb = persist.tile([P, 8, DKV], FP16)
            wv_sb = persist.tile([P, 8, DKV], FP16)
            wo_sb = persist.tile([P, 2, D], FP16)
            idn_sb = persist.tile([P, P], FP16)

            # ---- head: exp-table warm + ones column ----
            warm = small_p.tile([P, 8], F32, tag="warm")
            nc.vector.memset(warm[0:1, :], 0.0)
            nc.scalar.activation(warm[0:1, :], warm[0:1, :], EXP, scale=0.0)
            nc.vector.memset(Vag[:, :, :, DK : DK + 1], 1.0)

            xk_t, xq_t, xv_t = {}, {}, {}

            def load_x(pool, store, xT, idx, width, name):
                t = pool.tile([P, 8, width], FP16, tag="x", name=f"{name}{idx}")
                nc.sync.dma_start(
                    t,
                    xT.rearrange("(ko p) i -> p ko i", p=P)[
                        :, :, idx * width : (idx + 1) * width
                    ],
                )
                store[idx] = t

            # DMA issue order = arrival priority (in-order SP queue).
            nc.sync.dma_start(wq_sb, wq.rearrange("(ko p) n -> p ko n", p=P))
            load_x(xq_p, xq_t, xqT, 0, 256, "xq")
            nc.sync.dma_start(wk_sb, wk.rearrange("(ko p) n -> p ko n", p=P))
            load_x(xq_p, xq_t, xqT, 1, 256, "xq")
            for e in range(0, 8):
                load_x(xk_p, xk_t, xkT, e, 256, "xk")
            nc.sync.dma_start(wv_sb, wv.rearrange("(ko p) n -> p ko n", p=P))
            load_x(xv_p, xv_t, xvT, 0, 512, "xv")
            load_x(xq_p, xq_t, xqT, 2, 256, "xq")
            load_x(xq_p, xq_t, xqT, 3, 256, "xq")
            load_x(xv_p, xv_t, xvT, 1, 512, "xv")
            load_x(xv_p, xv_t, xvT, 2, 512, "xv")
            load_x(xv_p, xv_t, xvT, 3, 512, "xv")
            load_x(xq_p, xq_t, xqT, 4, 256, "xq")
            load_x(xq_p, xq_t, xqT, 5, 256, "xq")
            load_x(xq_p, xq_t, xqT, 6, 256, "xq")
            load_x(xq_p, xq_t, xqT, 7, 256, "xq")
            nc.sync.dma_start(wo_sb, wo.rearrange("(c p) n -> p c n", p=P))
            nc.sync.dma_start(idn_sb, idn[:, :])

            # ---- piece emitters (all 256-wide sub-pieces for KT/QT) ----
            qk_open = {}

            def qk_sub(w_sb, x_t, OUT, c, e, part=2):
                """OUT[:, c, e*256:(e+1)*256] from x eighth e (2048 cycles);
                part=0/1 emit one 1024-cycle half of the k-accumulation."""
                key = (id(w_sb), c, e)
                if part in (0, 2):
                    qk_open[key] = wo_ps.tile([P, 512], F32, tag="wo", name="qk")
                ps = qk_open[key]
                ks = range(8) if part == 2 else range(4 * part, 4 * part + 4)
                for k in ks:
                    nc.tensor.matmul(
                        ps[:, 0:256],
                        w_sb[:, k, c * P : (c + 1) * P],
                        x_t[e][:, k, :],
                        start=(k == 0),
                        stop=(k == 7),
                    )
                if part in (1, 2):
                    nc.vector.tensor_copy(
                        OUT[:, c, e * 256 : (e + 1) * 256], ps[:, 0:256]
                    )
                    del qk_open[key]

            def v_piece(jc, half):
                """Vag[:, 2*half:2*half+2, jc, 0:64] (1024 cycles)."""
                ps = wo_ps.tile([P, 512], F32, tag="wo", name="v")
                xt = xv_t[jc // 4]
                j0 = (jc % 4) * P
                for k in range(8):
                    nc.tensor.matmul(
                        ps[:, 0:P],
                        xt[:, k, j0 : j0 + P],
                        wv_sb[:, k, half * P : (half + 1) * P],
                        start=(k == 0),
                        stop=(k == 7),
                    )
                nc.vector.tensor_copy(
                    Vag[:, 2 * half : 2 * half + 2, jc, 0:DK],
                    ps[:, 0:P].rearrange("p (h d) -> p h d", h=2),
                )

            stages = {}
            wo_done = {}

            wo_open = {}

            def wo_piece(ib, eh, part=2):
                if ib not in stages:
                    stages[ib] = stage_p.tile([P, D], F32, tag="st", name=f"st{ib}")
                if part in (0, 2):
                    wo_open[(ib, eh)] = wo_ps.tile([P, 512], F32, tag="wo",
                                                   name="woo")
                ps = wo_open[(ib, eh)]
                cs = range(2) if part == 2 else [part]
                for c in cs:
                    nc.tensor.matmul(
                        ps,
                        ctxT[:, c, ib * P : (ib + 1) * P],
                        wo_sb[:, c, eh * 512 : (eh + 1) * 512],
                        start=(c == 0),
                        stop=(c == 1),
                    )
                if part in (1, 2):
                    del wo_open[(ib, eh)]
                    st = stages[ib]
                    nc.vector.tensor_copy(st[:, eh * 512 : (eh + 1) * 512], ps)
                    done = wo_done.setdefault(ib, set())
                    done.add(eh)
                    if done == {0, 1}:
                        nc.sync.dma_start(out[ib * P : (ib + 1) * P, :], st)

            # Last quarter: wo split into an early c0 half (staged to SBUF)
            # and a small tail c1 half + add, to shorten the tail chain.
            def wo_c0_piece(ib, eh):
                if ib not in stages:
                    stages[ib] = q3st_p.tile([P, D], F32, tag="q3st",
                                             name=f"q3st{ib}")
                ps = wo_ps.tile([P, 512], F32, tag="wo", name="woa")
                nc.tensor.matmul(
                    ps,
                    ctxT[:, 0, ib * P : (ib + 1) * P],
                    wo_sb[:, 0, eh * 512 : (eh + 1) * 512],
                    start=True,
                    stop=True,
                )
                eng = nc.vector
                eng.tensor_copy(stages[ib][:, eh * 512 : (eh + 1) * 512], ps)

            wo_c1_ps = {}

            def wo_c1_piece(ib, eh):
                if ib not in wo_c1_ps:
                    wo_c1_ps[ib] = sc_ps.tile([P, 1024], F32, tag="sc", name="wob")
                ps = wo_c1_ps[ib][:, eh * 512 : (eh + 1) * 512]
                nc.tensor.matmul(
                    ps,
                    ctxT[:, 1, ib * P : (ib + 1) * P],
                    wo_sb[:, 1, eh * 512 : (eh + 1) * 512],
                    start=True,
                    stop=True,
                )
                st = stages[ib]
                sl = st[:, eh * 512 : (eh + 1) * 512]
                eng = nc.vector
                eng.scalar_tensor_tensor(
                    sl, ps, 1.0, sl, mybir.AluOpType.mult, mybir.AluOpType.add
                )
                nc.sync.dma_start(
                    out[ib * P : (ib + 1) * P, eh * 512 : (eh + 1) * 512], sl
                )

            cx_tiles = {}
            at_tiles = {}

            def scores_exp(s, i2):
                q, h = (s % 8) // 2, 2 * (s // 8) + (s % 2)
                c, r0 = h // 2, (h % 2) * DK
                sc = sc_ps.tile([P, 1024], F32, tag="sc", name="sc")
                for jj in range(2):
                    jc = 2 * i2 + jj
                    nc.tensor.matmul(
                        sc[:, jj * 512 : (jj + 1) * 512],
                        KT[r0 : r0 + DK, c, jc * P : (jc + 1) * P],
                        QT[r0 : r0 + DK, c, q * 512 : (q + 1) * 512],
                        start=True,
                        stop=True,
                    )
                at = at_p.tile([P, 1024], FP16, tag="at", name=f"at{s}_{i2}")
                nc.scalar.activation(at, sc, EXP, scale=0.125)
                at_tiles[(s, i2)] = at

            def ctx_mm(s, i2):
                q, h = (s % 8) // 2, 2 * (s // 8) + (s % 2)
                at = at_tiles.pop((s, i2))
                cx = cx_tiles[s]
                for jj in range(2):
                    jc = 2 * i2 + jj
                    for ib in range(4):
                        nc.tensor.matmul(
                            cx[:, ib, :],
                            at[:, jj * 512 + ib * P : jj * 512 + (ib + 1) * P],
                            Vag[:, h, jc, :],
                            start=False,
                            stop=(i2 == 7 and jj == 1),
                            skip_group_check=True,
                        )

            def norm_sweep(s):
                q, h = (s % 8) // 2, 2 * (s // 8) + (s % 2)
                cx = cx_tiles.pop(s)
                second = h % 2 == 1  # both heads of chunk h//2 done
                rc = small_p.tile([P, 4], F32, tag="rc")
                with nc.allow_low_precision("softmax denom"):
                    nc.vector.reciprocal(
                        rc, cx[:, :, DK : DK + 1].rearrange("p a b -> p (a b)")
                    )
                for ib in range(4):
                    nc.vector.tensor_scalar_mul(
                        ctx_sb[:, q * 4 + ib, h * DK : (h + 1) * DK],
                        cx[:, ib, 0:DK],
                        rc[:, ib : ib + 1],
                    )
                    if second and s != 15:
                        nc.sync.dma_start_transpose(
                            ctxT[:, h // 2, (q * 4 + ib) * P : (q * 4 + ib + 1) * P],
                            ctx_sb[:, q * 4 + ib, (h // 2) * P : (h // 2 + 1) * P],
                        )
                if s == 15:  # tail: PE transposes (no HWDGE serialization)
                    for ib in range(4):
                        tp = wo_ps.tile([P, P], FP16, tag="wo", name="tp")
                        nc.tensor.transpose(
                            tp, ctx_sb[:, 12 + ib, P : 2 * P], idn_sb
                        )
                        eng = nc.vector
                        eng.tensor_copy(
                            ctxT[:, 1, (12 + ib) * P : (13 + ib) * P], tp
                        )

            # ---- weave units: (deadline, ready, cycles, fn) ----
            weaves = []
            # KT c0 eighths 1..7 (e0 in head): deadline iter 2e (pair-0).
            kt0_rdy = [0, 0, 1, 3, 4, 5, 7, 8]
            for e in range(1, 8):
                for part in range(2):
                    weaves.append(
                        (2 * e - 1, kt0_rdy[e], 1024,
                         lambda e=e, part=part: qk_sub(wk_sb, xk_t, KT, 0, e, part))
                    )
            # KT c1 eighths: needed from iter 64; weave in 20..60.
            for e in range(8):
                for part in range(2):
                    weaves.append(
                        (40 + 2 * e, 14 + e, 1024,
                         lambda e=e, part=part: qk_sub(wk_sb, xk_t, KT, 1, e, part))
                    )
            # QT c0 eighths 2..7 (e0,e1 in head): (c0, qi) by iter 16*qi.
            for e in range(2, 8):
                qi = e // 2
                for part in range(2):
                    weaves.append(
                        (16 * qi - 1, {2: 13, 3: 14, 4: 25, 5: 26, 6: 27, 7: 29}[e],
                         1024, lambda e=e, part=part: qk_sub(wq_sb, xq_t, QT, 0, e, part))
                    )
            # QT c1 eighths: by iter 64 + 16*qi; weave 22..60.
            for e in range(8):
                qi = e // 2
                for part in range(2):
                    weaves.append(
                        (63 + 16 * qi, 22 + e if e < 4 else 25 + e, 1024,
                         lambda e=e, part=part: qk_sub(wq_sb, xq_t, QT, 1, e, part))
                    )
            # V half-0 (heads 0,1): V[jc] by ctx of pair-0 at iter 2*(jc//2)+12.
            for jc in range(16):
                rdy = [10, 16, 19, 21][jc // 4]
                weaves.append(
                    (max(2 * (jc // 2) + 11, rdy), rdy, 1024,
                     lambda jc=jc: v_piece(jc, 0), ("v", jc, 0))
                )
            # V half-1 (heads 2,3): needed from iter ~64; weave 24..60.
            for jc in range(16):
                weaves.append(
                    (62 + 2 * (jc // 2), 18 + jc // 2, 1024,
                     lambda jc=jc: v_piece(jc, 1), ("v", jc, 1))
                )
            # wo: quarter q ready after norm of sweep 8+2q+1 (+transposes).
            for q in range(3):
                s_done = 8 + 2 * q + 1
                rdy = cit[(s_done, 7)] + 4
                for ib4 in range(4):
                    for eh in range(2):
                        for part in range(2):
                            weaves.append(
                                (min(rdy + 10, 127), rdy, 512,
                                 lambda q=q, ib4=ib4, eh=eh, part=part:
                                     wo_piece(q * 4 + ib4, eh, part))
                            )
            # q3: c0 halves early (ctxT c0 ready after sweep 7 + transposes),
            # c1 halves + add + store in the tail.
            for ib4 in range(4):
                for eh in range(2):
                    weaves.append(
                        (80, cit[(7, 7)] + 4, 512,
                         lambda ib4=ib4, eh=eh: wo_c0_piece(12 + ib4, eh))
                    )
                    weaves.append(
                        (200 + 2 * ib4 + eh, 131, 512,
                         lambda ib4=ib4, eh=eh: wo_c1_piece(12 + ib4, eh))
                    )
            weaves = [w if len(w) == 5 else (*w, None) for w in weaves]
            weaves.sort(key=lambda u: u[0])
            pending = list(weaves)

            def dummies(n):
                """n x 256 dummy rows to keep the PE p-state ramp alive
                while the head waits on input DMAs."""
                dps = wo_ps.tile([P, 512], F32, tag="wo", name="warmmm")
                for _ in range(n):
                    nc.tensor.matmul(
                        dps[:, 0:256], dmy, dmy2, start=True, stop=True
                    )

            with nc.named_scope("head"):
                dmy = dmy_p.tile([P, P], FP16, tag="dmy")
                dmy2 = dmy_p.tile([P, 256], FP16, tag="dmy2")
                nc.vector.memset(dmy, 0.0)
                nc.vector.memset(dmy2, 0.0)
                dummies(13)  # ramp to full clock by ~2.9us
                qk_sub(wq_sb, xq_t, QT, 0, 0)
                dummies(8)  # bridge to xq e1 arrival ~7.7us
                qk_sub(wq_sb, xq_t, QT, 0, 1)
                dummies(4)  # bridge to xk e0 arrival ~9.2us
                qk_sub(wk_sb, xk_t, KT, 0, 0)

            debt = 0
            for t in range(131):
                with nc.named_scope(f"it{t}"):
                    used = 0
                    if t in scores_at:
                        s, i2 = scores_at[t]
                        if i2 == 0:
                            cx_tiles[s] = cx_ps.tile(
                                [P, 4, DK + 1], F32, tag="cx", name=f"cx{s}"
                            )
                            nc.vector.memset(cx_tiles[s], 0.0)
                        scores_exp(s, i2)
                        used += 1024
                    for key in ctx_at.get(t, []):
                        s2k, i22k = key
                        h2k = 2 * (s2k // 8) + (s2k % 2)
                        need = {("v", 2 * i22k, h2k // 2),
                                ("v", 2 * i22k + 1, h2k // 2)}
                        for i in range(len(pending) - 1, -1, -1):
                            if pending[i][4] in need:
                                ent = pending.pop(i)
                                ent[3]()
                                used += ent[2]
                        ctx_mm(*key)
                        used += 520
                        if key in norm_after:
                            norm_sweep(norm_after[key])
                    if t >= 128:
                        used = -(10**9)
                    while pending:
                        idx = None
                        for i, (dl, rdy, cyc, fn, wkey) in enumerate(pending):
                            debt_after = max(0, debt + used + cyc - CAP)
                            if rdy <= t and (debt_after <= MAX_DEBT or dl <= t):
                                idx = i
                                break
                        if idx is None:
                            break
                        dl, rdy, cyc, fn, wkey = pending.pop(idx)
                        fn()
                        used += cyc
                    debt = max(0, debt + used - CAP) if t < 128 else 0
            with nc.named_scope("tail"):
                for ent in pending:
                    ent[3]()
    nc.compile()
    return nc


def get_nc():
    if not _NC_CACHE:
        _NC_CACHE.append(_build_nc())
    return _NC_CACHE[0]


def kernel(query, key, value, mask, Wq, Wk, Wv, Wo, **_run_kwargs):
    query = np.asarray(query, np.float32)
    key = np.asarray(key, np.float32)
    value = np.asarray(value, np.float32)
    Wq = np.asarray(Wq, np.float32)
    Wk = np.asarray(Wk, np.float32)
    Wv = np.asarray(Wv, np.float32)
    Wo = np.asarray(Wo, np.float32)

    nc = get_nc()
    f16 = np.float16
    in_maps = []
    for b in range(2):
        xqTb = np.ascontiguousarray(query[b].T).astype(f16)
        xkTb = np.ascontiguousarray(key[b].T).astype(f16)
        xvTb = np.ascontiguousarray(value[b].T).astype(f16)
        for g in range(4):
            c0 = g * DKV
            in_maps.append(
                {
                    "xqT": xqTb,
                    "xkT": xkTb,
                    "xvT": xvTb,
                    "wq": np.ascontiguousarray(Wq[:, c0 : c0 + DKV]).astype(f16),
                    "wk": np.ascontiguousarray(Wk[:, c0 : c0 + DKV]).astype(f16),
                    "wv": np.ascontiguousarray(Wv[:, c0 : c0 + DKV]).astype(f16),
                    "wo": np.ascontiguousarray(Wo[c0 : c0 + DKV, :]).astype(f16),
                    "idn": np.eye(P, dtype=f16),
                }
            )
    res = run_bass_kernel_spmd(nc, in_maps, core_ids=list(range(8)), **_run_kwargs)
    outs = [r["out"] for r in res.results]
    full = np.stack(
        [
            outs[0] + outs[1] + outs[2] + outs[3],
            outs[4] + outs[5] + outs[6] + outs[7],
        ]
    ).astype(np.float32)
    if _run_kwargs:
        return full, res
    return full


# revision 6
# speedup vs baseline: 1.0185x; 1.0054x over previous
"""Multi-head attention TRN2 kernel, v2.

Full inputs -> 8-core shard (batch x head-group) -> Bass/Tile kernel -> host
gather+reduce.  Problem: B=2, S=2048, D=1024, H=16, Dk=64, fp32, mask=0.

Core c = b*4 + g handles batch b, heads 4g..4g+3.  All intermediates fp16.

Engine plan (cost-model driven):
  PE    projections (full-128 contraction), scoresT (K=64), ctx with the
        attn chunk STATIONARY and V MOVING (65 rows incl. a ones column so
        the softmax denominator rides along), Wo.
  Act   exp only: one [128,1024] activation per iteration (two j-chunks
        side by side), scale=0.125 folded in.  This stream (~134us busy)
        is the bottleneck; everything else hides under it.
  DVE   psum->sbuf copies + reciprocals.
  Pool  softmax normalize + half the psum->sbuf drains.
  DMA   input stream, ctx transposes via the XBAR, output writes.

Sweeps (i-quarter q, head h) are ordered h-pair-outer: heads 0/1 for all
quarters first (sweeps 0..7 = q0h0,q0h1,q1h0,...), then heads 2/3
(sweeps 8..15).  KT/QT chunk-0 thus feeds the first 8 sweeps and chunk-1
streams in later.  Sweeps 0 and 1 are jc-interleaved into one 16-slot
stretch so the fresh-KT demand rate stays under the HBM stream rate.
ctx matmuls run a few iterations behind their exp (explicit CIT map);
V/QT/KT/Wo pieces fill PE slack via a deadline-greedy budgeter.
PSUM: scores ring 2x2 banks, ctx accums 2x1 bank, wo/proj ring 2x1 bank.
"""

import sys

import numpy as np

try:
    import concourse.bass as bass  # noqa: F401
except ImportError:  # harness runs from a bare directory
    sys.path.insert(0, "/opt/trn_rl_repo")
    import concourse.bass as bass  # noqa: F401

import concourse.tile as tile
from concourse import bacc, mybir
from concourse.bass_utils import run_bass_kernel_spmd

S = 2048
D = 1024
HG = 4  # heads per core
DK = 64
DKV = HG * DK  # 256
P = 128
F32 = mybir.dt.float32
FP16 = mybir.dt.float16
EXP = mybir.ActivationFunctionType.Exp

_NC_CACHE = []

AT_RING = 15
CAP = 2400  # PE cycles per Act period less per-instr overheads
MAX_DEBT = 1200  # PE-behind allowance absorbed by the sc ring


def _iteration_maps():
    """SIT: (s, i2) -> scores iteration; CIT: ctx iteration; both 0-based
    over 128 slots.  Sweeps 0/1 are interleaved over slots 0..15."""
    sit = {}
    for s in range(16):
        for i2 in range(8):
            if s < 2:
                sit[(s, i2)] = 2 * i2 + s
            else:
                sit[(s, i2)] = 8 * s + i2

    lag = {0: 13, 1: 13, 2: 13, 3: 12, 4: 11, 5: 10, 6: 9, 7: 8,
           8: 7, 9: 6, 10: 5, 11: 4, 12: 3, 13: 2, 14: 2, 15: 1}
    cit = {}
    for s in range(16):
        L = lag.get(s, 2)
        for i2 in range(8):
            cit[(s, i2)] = sit[(s, i2)] + L
    return sit, cit


def _build_nc():
    nc = bacc.Bacc("TRN2", target_bir_lowering=False, debug=False)
    xqT = nc.dram_tensor("xqT", [D, S], FP16, kind="ExternalInput")
    xkT = nc.dram_tensor("xkT", [D, S], FP16, kind="ExternalInput")
    xvT = nc.dram_tensor("xvT", [D, S], FP16, kind="ExternalInput")
    wq = nc.dram_tensor("wq", [D, DKV], FP16, kind="ExternalInput")
    wk = nc.dram_tensor("wk", [D, DKV], FP16, kind="ExternalInput")
    wv = nc.dram_tensor("wv", [D, DKV], FP16, kind="ExternalInput")
    wo = nc.dram_tensor("wo", [DKV, D], FP16, kind="ExternalInput")
    idn = nc.dram_tensor("idn", [P, P], FP16, kind="ExternalInput")
    out = nc.dram_tensor("out", [S, D], F32, kind="ExternalOutput")

    sit, cit = _iteration_maps()
    scores_at = {}  # iter -> (s, i2)
    for k, t in sit.items():
        scores_at[t] = k
    ctx_at = {}
    for (s, i2), t in cit.items():
        ctx_at.setdefault(t, []).append((s, i2))
        ctx_at[t].sort(key=lambda k: (cit[k], k))
    norm_after = {}  # (s, i2) -> s to normalize right after that ctx
    for s in range(16):
        norm_after[(s, 7)] = s

    with tile.TileContext(nc) as tc:
        with (
            tc.tile_pool(name="persist", bufs=1) as persist,
            tc.tile_pool(name="xk_p", bufs=8) as xk_p,
            tc.tile_pool(name="xq_p", bufs=8) as xq_p,
            tc.tile_pool(name="xv_p", bufs=4) as xv_p,
            tc.tile_pool(name="at_p", bufs=AT_RING) as at_p,
            tc.tile_pool(name="stage", bufs=2) as stage_p,
            tc.tile_pool(name="q3st", bufs=4) as q3st_p,
            tc.tile_pool(name="small", bufs=4) as small_p,
            tc.tile_pool(name="dmy", bufs=1) as dmy_p,
            tc.tile_pool(name="sc_ps", bufs=2, space="PSUM") as sc_ps,
            tc.tile_pool(name="cx_ps", bufs=2, space="PSUM") as cx_ps,
            tc.tile_pool(name="wo_ps", bufs=2, space="PSUM") as wo_ps,
        ):
            # ---- persistent SBUF ----
            QT = persist.tile([P, 2, S], FP16)  # [(h%2)*64+dk, h//2, i]
            KT = persist.tile([P, 2, S], FP16)
            Vag = persist.tile([P, HG, 16, DK + 1], FP16)  # [j%128, h, jc, dk|1]
            ctx_sb = persist.tile([P, 16, DKV], FP16)  # [i%128, ib, dkv]
            ctxT = persist.tile([P, 2, S], FP16)  # [dkv%128, dkv//128, i]
            wq_sb = persist.tile([P, 8, DKV], FP16)
            wk_sb = persist.tile([P, 8, DKV], FP16)
            wv_sb = persist.tile([P, 8, DKV], FP16)
            wo_sb = persist.tile([P, 2, D], FP16)
            idn_sb = persist.tile([P, P], FP16)

            # ---- head: exp-table warm + ones column ----
            warm = small_p.tile([P, 8], F32, tag="warm")
            nc.vector.memset(warm[0:1, :], 0.0)
            nc.scalar.activation(warm[0:1, :], warm[0:1, :], EXP, scale=0.0)
            nc.vector.memset(Vag[:, :, :, DK : DK + 1], 1.0)

            xk_t, xq_t, xv_t = {}, {}, {}

            def load_x(pool, store, xT, idx, width, name):
                t = pool.tile([P, 8, width], FP16, tag="x", name=f"{name}{idx}")
                nc.sync.dma_start(
                    t,
                    xT.rearrange("(ko p) i -> p ko i", p=P)[
                        :, :, idx * width : (idx + 1) * width
                    ],
                )
                store[idx] = t

            # DMA issue order = arrival priority (in-order SP queue).
            nc.sync.dma_start(wq_sb, wq.rearrange("(ko p) n -> p ko n", p=P))
            load_x(xq_p, xq_t, xqT, 0, 256, "xq")
            nc.sync.dma_start(wk_sb, wk.rearrange("(ko p) n -> p ko n", p=P))
            load_x(xq_p, xq_t, xqT, 1, 256, "xq")
            for e in range(0, 8):
                load_x(xk_p, xk_t, xkT, e, 256, "xk")
            nc.sync.dma_start(wv_sb, wv.rearrange("(ko p) n -> p ko n", p=P))
            load_x(xv_p, xv_t, xvT, 0, 512, "xv")
            load_x(xq_p, xq_t, xqT, 2, 256, "xq")
            load_x(xq_p, xq_t, xqT, 3, 256, "xq")
            load_x(xv_p, xv_t, xvT, 1, 512, "xv")
            load_x(xv_p, xv_t, xvT, 2, 512, "xv")
            load_x(xv_p, xv_t, xvT, 3, 512, "xv")
            load_x(xq_p, xq_t, xqT, 4, 256, "xq")
            load_x(xq_p, xq_t, xqT, 5, 256, "xq")
            load_x(xq_p, xq_t, xqT, 6, 256, "xq")
            load_x(xq_p, xq_t, xqT, 7, 256, "xq")
            nc.sync.dma_start(wo_sb, wo.rearrange("(c p) n -> p c n", p=P))
            nc.sync.dma_start(idn_sb, idn[:, :])

            # ---- piece emitters (all 256-wide sub-pieces for KT/QT) ----
            qk_open = {}

            def qk_sub(w_sb, x_t, OUT, c, e, part=2):
                """OUT[:, c, e*256:(e+1)*256] from x eighth e (2048 cycles);
                part=0/1 emit one 1024-cycle half of the k-accumulation."""
                key = (id(w_sb), c, e)
                if part in (0, 2):
                    qk_open[key] = wo_ps.tile([P, 512], F32, tag="wo", name="qk")
                ps = qk_open[key]
                ks = range(8) if part == 2 else range(4 * part, 4 * part + 4)
                for k in ks:
                    nc.tensor.matmul(
                        ps[:, 0:256],
                        w_sb[:, k, c * P : (c + 1) * P],
                        x_t[e][:, k, :],
                        start=(k == 0),
                        stop=(k == 7),
                    )
                if part in (1, 2):
                    nc.vector.tensor_copy(
                        OUT[:, c, e * 256 : (e + 1) * 256], ps[:, 0:256]
                    )
                    del qk_open[key]

            def v_piece(jc, half):
                """Vag[:, 2*half:2*half+2, jc, 0:64] (1024 cycles)."""
                ps = wo_ps.tile([P, 512], F32, tag="wo", name="v")
                xt = xv_t[jc // 4]
                j0 = (jc % 4) * P
                for k in range(8):
                    nc.tensor.matmul(
                        ps[:, 0:P],
                        xt[:, k, j0 : j0 + P],
                        wv_sb[:, k, half * P : (half + 1) * P],
                        start=(k == 0),
                        stop=(k == 7),
                    )
                nc.vector.tensor_copy(
                    Vag[:, 2 * half : 2 * half + 2, jc, 0:DK],
                    ps[:, 0:P].rearrange("p (h d) -> p h d", h=2),
                )

            stages = {}
            wo_done = {}

            wo_open = {}

            def wo_piece(ib, eh, part=2):
                if ib not in stages:
                    stages[ib] = stage_p.tile([P, D], F32, tag="st", name=f"st{ib}")
                if part in (0, 2):
                    wo_open[(ib, eh)] = wo_ps.tile([P, 512], F32, tag="wo",
                                                   name="woo")
                ps = wo_open[(ib, eh)]
                cs = range(2) if part == 2 else [part]
                for c in cs:
                    nc.tensor.matmul(
                        ps,
                        ctxT[:, c, ib * P : (ib + 1) * P],
                        wo_sb[:, c, eh * 512 : (eh + 1) * 512],
                        start=(c == 0),
                        stop=(c == 1),
                    )
                if part in (1, 2):
                    del wo_open[(ib, eh)]
                    st = stages[ib]
                    nc.vector.tensor_copy(st[:, eh * 512 : (eh + 1) * 512], ps)
                    done = wo_done.setdefault(ib, set())
                    done.add(eh)
                    if done == {0, 1}:
                        nc.sync.dma_start(out[ib * P : (ib + 1) * P, :], st)

            # Last quarter: wo split into an early c0 half (staged to SBUF)
            # and a small tail c1 half + add, to shorten the tail chain.
            def wo_c0_piece(ib, eh):
                if ib not in stages:
                    stages[ib] = q3st_p.tile([P, D], F32, tag="q3st",
                                             name=f"q3st{ib}")
                ps = wo_ps.tile([P, 512], F32, tag="wo", name="woa")
                nc.tensor.matmul(
                    ps,
                    ctxT[:, 0, ib * P : (ib + 1) * P],
                    wo_sb[:, 0, eh * 512 : (eh + 1) * 512],
                    start=True,
                    stop=True,
                )
                eng = nc.vector
                eng.tensor_copy(stages[ib][:, eh * 512 : (eh + 1) * 512], ps)

            wo_c1_ps = {}

            def wo_c1_piece(ib, eh):
                if ib not in wo_c1_ps:
                    wo_c1_ps[ib] = sc_ps.tile([P, 1024], F32, tag="sc", name="wob")
                ps = wo_c1_ps[ib][:, eh * 512 : (eh + 1) * 512]
                nc.tensor.matmul(
                    ps,
                    ctxT[:, 1, ib * P : (ib + 1) * P],
                    wo_sb[:, 1, eh * 512 : (eh + 1) * 512],
                    start=True,
                    stop=True,
                )
                st = stages[ib]
                sl = st[:, eh * 512 : (eh + 1) * 512]
                eng = nc.vector
                eng.scalar_tensor_tensor(
                    sl, ps, 1.0, sl, mybir.AluOpType.mult, mybir.AluOpType.add
                )
                nc.sync.dma_start(
                    out[ib * P : (ib + 1) * P, eh * 512 : (eh + 1) * 512], sl
                )

            cx_tiles = {}
            at_tiles = {}

            def scores_exp(s, i2):
                q, h = (s % 8) // 2, 2 * (s // 8) + (s % 2)
                c, r0 = h // 2, (h % 2) * DK
                sc = sc_ps.tile([P, 1024], F32, tag="sc", name="sc")
                for jj in range(2):
                    jc = 2 * i2 + jj
                    nc.tensor.matmul(
                        sc[:, jj * 512 : (jj + 1) * 512],
                        KT[r0 : r0 + DK, c, jc * P : (jc + 1) * P],
                        QT[r0 : r0 + DK, c, q * 512 : (q + 1) * 512],
                        start=True,
                        stop=True,
                    )
                at = at_p.tile([P, 1024], FP16, tag="at", name=f"at{s}_{i2}")
                nc.scalar.activation(at, sc, EXP, scale=0.125)
                at_tiles[(s, i2)] = at

            def ctx_mm(s, i2):
                q, h = (s % 8) // 2, 2 * (s // 8) + (s % 2)
                at = at_tiles.pop((s, i2))
                cx = cx_tiles[s]
                for jj in range(2):
                    jc = 2 * i2 + jj
                    for ib in range(4):
                        nc.tensor.matmul(
                            cx[:, ib, :],
                            at[:, jj * 512 + ib * P : jj * 512 + (ib + 1) * P],
                            Vag[:, h, jc, :],
                            start=False,
                            stop=(i2 == 7 and jj == 1),
                            skip_group_check=True,
                        )

            def norm_sweep(s):
                q, h = (s % 8) // 2, 2 * (s // 8) + (s % 2)
                cx = cx_tiles.pop(s)
                second = h % 2 == 1  # both heads of chunk h//2 done
                rc = small_p.tile([P, 4], F32, tag="rc")
                with nc.allow_low_precision("softmax denom"):
                    nc.vector.reciprocal(
                        rc, cx[:, :, DK : DK + 1].rearrange("p a b -> p (a b)")
                    )
                for ib in range(4):
                    nc.vector.tensor_scalar_mul(
                        ctx_sb[:, q * 4 + ib, h * DK : (h + 1) * DK],
                        cx[:, ib, 0:DK],
                        rc[:, ib : ib + 1],
                    )
                    if second and s != 15:
                        nc.sync.dma_start_transpose(
                            ctxT[:, h // 2, (q * 4 + ib) * P : (q * 4 + ib + 1) * P],
                            ctx_sb[:, q * 4 + ib, (h // 2) * P : (h // 2 + 1) * P],
                        )
                if s == 15:  # tail: PE transposes (no HWDGE serialization)
                    for ib in range(4):
                        tp = wo_ps.tile([P, P], FP16, tag="wo", name="tp")
                        nc.tensor.transpose(
                            tp, ctx_sb[:, 12 + ib, P : 2 * P], idn_sb
                        )
                        eng = nc.vector
                        eng.tensor_copy(
                            ctxT[:, 1, (12 + ib) * P : (13 + ib) * P], tp
                        )

            # ---- weave units: (deadline, ready, cycles, fn) ----
            weaves = []
            # KT c0 eighths 1..7 (e0 in head): deadline iter 2e (pair-0).
            kt0_rdy = [0, 0, 1, 3, 4, 5, 7, 8]
            for e in range(1, 8):
                for part in range(2):
                    weaves.append(
                        (2 * e - 1, kt0_rdy[e], 1024,
                         lambda e=e, part=part: qk_sub(wk_sb, xk_t, KT, 0, e, part))
                    )
            # KT c1 eighths: needed from iter 64; weave in 20..60.
            for e in range(8):
                for part in range(2):
                    weaves.append(
                        (40 + 2 * e, 14 + e, 1024,
                         lambda e=e, part=part: qk_sub(wk_sb, xk_t, KT, 1, e, part))
                    )
            # QT c0 eighths 2..7 (e0,e1 in head): (c0, qi) by iter 16*qi.
            for e in range(2, 8):
                qi = e // 2
                for part in range(2):
                    weaves.append(
                        (16 * qi - 1, {2: 13, 3: 14, 4: 25, 5: 26, 6: 27, 7: 29}[e],
                         1024, lambda e=e, part=part: qk_sub(wq_sb, xq_t, QT, 0, e, part))
                    )
            # QT c1 eighths: by iter 64 + 16*qi; weave 22..60.
            for e in range(8):
                qi = e // 2
                for part in range(2):
                    weaves.append(
                        (63 + 16 * qi, 22 + e if e < 4 else 25 + e, 1024,
                         lambda e=e, part=part: qk_sub(wq_sb, xq_t, QT, 1, e, part))
                    )
            # V half-0 (heads 0,1): V[jc] by ctx of pair-0 at iter 2*(jc//2)+12.
            for jc in range(16):
                rdy = [10, 16, 19, 21][jc // 4]
                weaves.append(
                    (max(2 * (jc // 2) + 11, rdy), rdy, 1024,
                     lambda jc=jc: v_piece(jc, 0), ("v", jc, 0))
                )
            # V half-1 (heads 2,3): needed from iter ~64; weave 24..60.
            for jc in range(16):
                weaves.append(
                    (62 + 2 * (jc // 2), 18 + jc // 2, 1024,
                     lambda jc=jc: v_piece(jc, 1), ("v", jc, 1))
                )
            # wo: quarter q ready after norm of sweep 8+2q+1 (+transposes).
            for q in range(3):
                s_done = 8 + 2 * q + 1
                rdy = cit[(s_done, 7)] + 3
                for ib4 in range(4):
                    for eh in range(2):
                        for part in range(2):
                            weaves.append(
                                (min(rdy + 10, 127), rdy, 512,
                                 lambda q=q, ib4=ib4, eh=eh, part=part:
                                     wo_piece(q * 4 + ib4, eh, part))
                            )
            # q3: c0 halves early (ctxT c0 ready after sweep 7 + transposes),
            # c1 halves + add + store in the tail.
            for ib4 in range(4):
                for eh in range(2):
                    weaves.append(
                        (80, cit[(7, 7)] + 4, 512,
                         lambda ib4=ib4, eh=eh: wo_c0_piece(12 + ib4, eh))
                    )
                    weaves.append(
                        (200 + 2 * ib4 + eh, 131, 512,
                         lambda ib4=ib4, eh=eh: wo_c1_piece(12 + ib4, eh))
                    )
            weaves = [w if len(w) == 5 else (*w, None) for w in weaves]
            weaves.sort(key=lambda u: u[0])
            pending = list(weaves)

            def dummies(n):
                """n x 256 dummy rows to keep the PE p-state ramp alive
                while the head waits on input DMAs."""
                dps = wo_ps.tile([P, 512], F32, tag="wo", name="warmmm")
                for _ in range(n):
                    nc.tensor.matmul(
                        dps[:, 0:256], dmy, dmy2, start=True, stop=True
                    )

            with nc.named_scope("head"):
                dmy = dmy_p.tile([P, P], FP16, tag="dmy")
                dmy2 = dmy_p.tile([P, 256], FP16, tag="dmy2")
                nc.vector.memset(dmy, 0.0)
                nc.vector.memset(dmy2, 0.0)
                dummies(13)  # ramp to full clock by ~2.9us
                qk_sub(wq_sb, xq_t, QT, 0, 0)
                dummies(8)  # bridge to xq e1 arrival ~7.7us
                qk_sub(wq_sb, xq_t, QT, 0, 1)
                dummies(4)  # bridge to xk e0 arrival ~9.2us
                qk_sub(wk_sb, xk_t, KT, 0, 0)

            debt = 0
            for t in range(131):
                with nc.named_scope(f"it{t}"):
                    used = 0
                    if t in scores_at:
                        s, i2 = scores_at[t]
                        if i2 == 0:
                            cx_tiles[s] = cx_ps.tile(
                                [P, 4, DK + 1], F32, tag="cx", name=f"cx{s}"
                            )
                            nc.vector.memset(cx_tiles[s], 0.0)
                        scores_exp(s, i2)
                        used += 1024
                    for key in ctx_at.get(t, []):
                        s2k, i22k = key
                        h2k = 2 * (s2k // 8) + (s2k % 2)
                        need = {("v", 2 * i22k, h2k // 2),
                                ("v", 2 * i22k + 1, h2k // 2)}
                        for i in range(len(pending) - 1, -1, -1):
                            if pending[i][4] in need:
                                ent = pending.pop(i)
                                ent[3]()
                                used += ent[2]
                        ctx_mm(*key)
                        used += 520
                        if key in norm_after:
                            norm_sweep(norm_after[key])
                    if t >= 128:
                        used = -(10**9)
                    while pending:
                        idx = None
                        for i, (dl, rdy, cyc, fn, wkey) in enumerate(pending):
                            debt_after = max(0, debt + used + cyc - CAP)
                            if rdy <= t and (debt_after <= MAX_DEBT or dl <= t):
                                idx = i
                                break
                        if idx is None:
                            break
                        dl, rdy, cyc, fn, wkey = pending.pop(idx)
                        fn()
                        used += cyc
                    debt = max(0, debt + used - CAP) if t < 128 else 0
            with nc.named_scope("tail"):
                for ent in pending:
                    ent[3]()
    nc.compile()
    return nc


def get_nc():
    if not _NC_CACHE:
        _NC_CACHE.append(_build_nc())
    return _NC_CACHE[0]


def kernel(query, key, value, mask, Wq, Wk, Wv, Wo, **_run_kwargs):
    query = np.asarray(query, np.float32)
    key = np.asarray(key, np.float32)
    value = np.asarray(value, np.float32)
    Wq = np.asarray(Wq, np.float32)
    Wk = np.asarray(Wk, np.float32)
    Wv = np.asarray(Wv, np.float32)
    Wo = np.asarray(Wo, np.float32)

    nc = get_nc()
    f16 = np.float16
    in_maps = []
    for b in range(2):
        xqTb = np.ascontiguousarray(query[b].T).astype(f16)
        xkTb = np.ascontiguousarray(key[b].T).astype(f16)
        xvTb = np.ascontiguousarray(value[b].T).astype(f16)
        for g in range(4):
            c0 = g * DKV
            in_maps.append(
                {
                    "xqT": xqTb,
                    "xkT": xkTb,
                    "xvT": xvTb,
                    "wq": np.ascontiguousarray(Wq[:, c0 : c0 + DKV]).astype(f16),
                    "wk": np.ascontiguousarray(Wk[:, c0 : c0 + DKV]).astype(f16),
                    "wv": np.ascontiguousarray(Wv[:, c0 : c0 + DKV]).astype(f16),
                    "wo": np.ascontiguousarray(Wo[c0 : c0 + DKV, :]).astype(f16),
                    "idn": np.eye(P, dtype=f16),
                }
            )
    res = run_bass_kernel_spmd(nc, in_maps, core_ids=list(range(8)), **_run_kwargs)
    outs = [r["out"] for r in res.results]
    full = np.stack(
        [
            outs[0] + outs[1] + outs[2] + outs[3],
            outs[4] + outs[5] + outs[6] + outs[7],
        ]
    ).astype(np.float32)
    if _run_kwargs:
        return full, res
    return full


# revision 7
# speedup vs baseline: 1.0404x; 1.0214x over previous
"""Multi-head attention TRN2 kernel, v2.

Full inputs -> 8-core shard (batch x head-group) -> Bass/Tile kernel -> host
gather+reduce.  Problem: B=2, S=2048, D=1024, H=16, Dk=64, fp32, mask=0.

Core c = b*4 + g handles batch b, heads 4g..4g+3.  All intermediates fp16.

Engine plan (cost-model driven):
  PE    projections (full-128 contraction), scoresT (K=64), ctx with the
        attn chunk STATIONARY and V MOVING (65 rows incl. a ones column so
        the softmax denominator rides along), Wo.
  Act   exp only: one [128,1024] activation per iteration (two j-chunks
        side by side), scale=0.125 folded in.  This stream (~134us busy)
        is the bottleneck; everything else hides under it.
  DVE   psum->sbuf copies + reciprocals.
  Pool  softmax normalize + half the psum->sbuf drains.
  DMA   input stream, ctx transposes via the XBAR, output writes.

Sweeps (i-quarter q, head h) are ordered h-pair-outer: heads 0/1 for all
quarters first (sweeps 0..7 = q0h0,q0h1,q1h0,...), then heads 2/3
(sweeps 8..15).  KT/QT chunk-0 thus feeds the first 8 sweeps and chunk-1
streams in later.  Sweeps 0 and 1 are jc-interleaved into one 16-slot
stretch so the fresh-KT demand rate stays under the HBM stream rate.
ctx matmuls run a few iterations behind their exp (explicit CIT map);
V/QT/KT/Wo pieces fill PE slack via a deadline-greedy budgeter.
PSUM: scores ring 2x2 banks, ctx accums 2x1 bank, wo/proj ring 2x1 bank.
"""

import sys

import numpy as np

try:
    import concourse.bass as bass  # noqa: F401
except ImportError:  # harness runs from a bare directory
    sys.path.insert(0, "/opt/trn_rl_repo")
    import concourse.bass as bass  # noqa: F401

import concourse.tile as tile
from concourse import bacc, mybir
from concourse.bass_utils import run_bass_kernel_spmd

S = 2048
D = 1024
HG = 4  # heads per core
DK = 64
DKV = HG * DK  # 256
P = 128
F32 = mybir.dt.float32
FP16 = mybir.dt.float16
EXP = mybir.ActivationFunctionType.Exp

_NC_CACHE = []

AT_RING = 15
CAP = 2400  # PE cycles per Act period less per-instr overheads
MAX_DEBT = 1200  # PE-behind allowance absorbed by the sc ring


def _iteration_maps():
    """SIT: (s, i2) -> scores iteration; CIT: ctx iteration; both 0-based
    over 128 slots.  Sweeps 0/1 are interleaved over slots 0..15."""
    sit = {}
    for s in range(16):
        for i2 in range(8):
            if s < 2:
                sit[(s, i2)] = 2 * i2 + s
            else:
                sit[(s, i2)] = 8 * s + i2

    lag = {0: 13, 1: 13, 2: 13, 3: 12, 4: 11, 5: 10, 6: 9, 7: 8,
           8: 7, 9: 6, 10: 5, 11: 4, 12: 3, 13: 2, 14: 2, 15: 1}
    cit = {}
    for s in range(16):
        L = lag.get(s, 2)
        for i2 in range(8):
            cit[(s, i2)] = sit[(s, i2)] + L
    return sit, cit


def _build_nc():
    nc = bacc.Bacc("TRN2", target_bir_lowering=False, debug=False)
    xqT = nc.dram_tensor("xqT", [D, S], FP16, kind="ExternalInput")
    xkT = nc.dram_tensor("xkT", [D, S], FP16, kind="ExternalInput")
    xvT = nc.dram_tensor("xvT", [D, S], FP16, kind="ExternalInput")
    wq = nc.dram_tensor("wq", [D, DKV], FP16, kind="ExternalInput")
    wk = nc.dram_tensor("wk", [D, DKV], FP16, kind="ExternalInput")
    wv = nc.dram_tensor("wv", [D, DKV], FP16, kind="ExternalInput")
    wo = nc.dram_tensor("wo", [DKV, D], FP16, kind="ExternalInput")
    idn = nc.dram_tensor("idn", [P, P], FP16, kind="ExternalInput")
    out = nc.dram_tensor("out", [S, D], F32, kind="ExternalOutput")

    sit, cit = _iteration_maps()
    scores_at = {}  # iter -> (s, i2)
    for k, t in sit.items():
        scores_at[t] = k
    ctx_at = {}
    for (s, i2), t in cit.items():
        ctx_at.setdefault(t, []).append((s, i2))
        ctx_at[t].sort(key=lambda k: (cit[k], k))
    norm_after = {}  # (s, i2) -> s to normalize right after that ctx
    for s in range(16):
        norm_after[(s, 7)] = s

    with tile.TileContext(nc) as tc:
        with (
            tc.tile_pool(name="persist", bufs=1) as persist,
            tc.tile_pool(name="xk_p", bufs=8) as xk_p,
            tc.tile_pool(name="xq_p", bufs=8) as xq_p,
            tc.tile_pool(name="xv_p", bufs=4) as xv_p,
            tc.tile_pool(name="at_p", bufs=AT_RING) as at_p,
            tc.tile_pool(name="stage", bufs=2) as stage_p,
            tc.tile_pool(name="q3st", bufs=4) as q3st_p,
            tc.tile_pool(name="small", bufs=4) as small_p,
            tc.tile_pool(name="dmy", bufs=1) as dmy_p,
            tc.tile_pool(name="sc_ps", bufs=2, space="PSUM") as sc_ps,
            tc.tile_pool(name="cx_ps", bufs=2, space="PSUM") as cx_ps,
            tc.tile_pool(name="wo_ps", bufs=2, space="PSUM") as wo_ps,
        ):
            # ---- persistent SBUF ----
            QT = persist.tile([P, 2, S], FP16)  # [(h%2)*64+dk, h//2, i]
            KT = persist.tile([P, 2, S], FP16)
            Vag = persist.tile([P, HG, 16, DK + 1], FP16)  # [j%128, h, jc, dk|1]
            ctx_sb = persist.tile([P, 16, DKV], FP16)  # [i%128, ib, dkv]
            ctxT = persist.tile([P, 2, S], FP16)  # [dkv%128, dkv//128, i]
            wq_sb = persist.tile([P, 8, DKV], FP16)
            wk_sb = persist.tile([P, 8, DKV], FP16)
            wv_sb = persist.tile([P, 8, DKV], FP16)
            wo_sb = persist.tile([P, 2, D], FP16)
            idn_sb = persist.tile([P, P], FP16)

            # ---- head: exp-table warm + ones column ----
            warm = small_p.tile([P, 8], F32, tag="warm")
            nc.vector.memset(warm[0:1, :], 0.0)
            nc.scalar.activation(warm[0:1, :], warm[0:1, :], EXP, scale=0.0)
            nc.vector.memset(Vag[:, :, :, DK : DK + 1], 1.0)

            xk_t, xq_t, xv_t = {}, {}, {}

            def load_x(pool, store, xT, idx, width, name):
                t = pool.tile([P, 8, width], FP16, tag="x", name=f"{name}{idx}")
                nc.sync.dma_start(
                    t,
                    xT.rearrange("(ko p) i -> p ko i", p=P)[
                        :, :, idx * width : (idx + 1) * width
                    ],
                )
                store[idx] = t

            # DMA issue order = arrival priority (in-order SP queue).
            nc.sync.dma_start(wq_sb, wq.rearrange("(ko p) n -> p ko n", p=P))
            load_x(xq_p, xq_t, xqT, 0, 256, "xq")
            nc.sync.dma_start(wk_sb, wk.rearrange("(ko p) n -> p ko n", p=P))
            load_x(xq_p, xq_t, xqT, 1, 256, "xq")
            for e in range(0, 8):
                load_x(xk_p, xk_t, xkT, e, 256, "xk")
            nc.sync.dma_start(wv_sb, wv.rearrange("(ko p) n -> p ko n", p=P))
            load_x(xv_p, xv_t, xvT, 0, 512, "xv")
            load_x(xq_p, xq_t, xqT, 2, 256, "xq")
            load_x(xq_p, xq_t, xqT, 3, 256, "xq")
            load_x(xv_p, xv_t, xvT, 1, 512, "xv")
            load_x(xv_p, xv_t, xvT, 2, 512, "xv")
            load_x(xv_p, xv_t, xvT, 3, 512, "xv")
            load_x(xq_p, xq_t, xqT, 4, 256, "xq")
            load_x(xq_p, xq_t, xqT, 5, 256, "xq")
            load_x(xq_p, xq_t, xqT, 6, 256, "xq")
            load_x(xq_p, xq_t, xqT, 7, 256, "xq")
            nc.sync.dma_start(wo_sb, wo.rearrange("(c p) n -> p c n", p=P))
            nc.sync.dma_start(idn_sb, idn[:, :])

            # ---- piece emitters (all 256-wide sub-pieces for KT/QT) ----
            qk_open = {}

            def qk_sub(w_sb, x_t, OUT, c, e, part=2):
                """OUT[:, c, e*256:(e+1)*256] from x eighth e (2048 cycles);
                part=0/1 emit one 1024-cycle half of the k-accumulation."""
                key = (id(w_sb), c, e)
                if part in (0, 2):
                    qk_open[key] = wo_ps.tile([P, 512], F32, tag="wo", name="qk")
                ps = qk_open[key]
                ks = range(8) if part == 2 else range(4 * part, 4 * part + 4)
                for k in ks:
                    nc.tensor.matmul(
                        ps[:, 0:256],
                        w_sb[:, k, c * P : (c + 1) * P],
                        x_t[e][:, k, :],
                        start=(k == 0),
                        stop=(k == 7),
                    )
                if part in (1, 2):
                    nc.vector.tensor_copy(
                        OUT[:, c, e * 256 : (e + 1) * 256], ps[:, 0:256]
                    )
                    del qk_open[key]

            def v_piece(jc, half):
                """Vag[:, 2*half:2*half+2, jc, 0:64] (1024 cycles)."""
                ps = wo_ps.tile([P, 512], F32, tag="wo", name="v")
                xt = xv_t[jc // 4]
                j0 = (jc % 4) * P
                for k in range(8):
                    nc.tensor.matmul(
                        ps[:, 0:P],
                        xt[:, k, j0 : j0 + P],
                        wv_sb[:, k, half * P : (half + 1) * P],
                        start=(k == 0),
                        stop=(k == 7),
                    )
                nc.vector.tensor_copy(
                    Vag[:, 2 * half : 2 * half + 2, jc, 0:DK],
                    ps[:, 0:P].rearrange("p (h d) -> p h d", h=2),
                )

            stages = {}
            wo_done = {}

            wo_open = {}

            def wo_piece(ib, eh, part=2):
                if ib not in stages:
                    stages[ib] = stage_p.tile([P, D], F32, tag="st", name=f"st{ib}")
                if part in (0, 2):
                    wo_open[(ib, eh)] = wo_ps.tile([P, 512], F32, tag="wo",
                                                   name="woo")
                ps = wo_open[(ib, eh)]
                cs = range(2) if part == 2 else [part]
                for c in cs:
                    nc.tensor.matmul(
                        ps,
                        ctxT[:, c, ib * P : (ib + 1) * P],
                        wo_sb[:, c, eh * 512 : (eh + 1) * 512],
                        start=(c == 0),
                        stop=(c == 1),
                    )
                if part in (1, 2):
                    del wo_open[(ib, eh)]
                    st = stages[ib]
                    nc.vector.tensor_copy(st[:, eh * 512 : (eh + 1) * 512], ps)
                    done = wo_done.setdefault(ib, set())
                    done.add(eh)
                    if done == {0, 1}:
                        nc.sync.dma_start(out[ib * P : (ib + 1) * P, :], st)

            # Last quarter: wo split into an early c0 half (staged to SBUF)
            # and a small tail c1 half + add, to shorten the tail chain.
            def wo_c0_piece(ib, eh):
                if ib not in stages:
                    stages[ib] = q3st_p.tile([P, D], F32, tag="q3st",
                                             name=f"q3st{ib}")
                ps = wo_ps.tile([P, 512], F32, tag="wo", name="woa")
                nc.tensor.matmul(
                    ps,
                    ctxT[:, 0, ib * P : (ib + 1) * P],
                    wo_sb[:, 0, eh * 512 : (eh + 1) * 512],
                    start=True,
                    stop=True,
                )
                eng = nc.vector
                eng.tensor_copy(stages[ib][:, eh * 512 : (eh + 1) * 512], ps)

            wo_c1_ps = {}

            def wo_c1_piece(ib, eh):
                if ib not in wo_c1_ps:
                    wo_c1_ps[ib] = sc_ps.tile([P, 1024], F32, tag="sc", name="wob")
                ps = wo_c1_ps[ib][:, eh * 512 : (eh + 1) * 512]
                nc.tensor.matmul(
                    ps,
                    ctxT[:, 1, ib * P : (ib + 1) * P],
                    wo_sb[:, 1, eh * 512 : (eh + 1) * 512],
                    start=True,
                    stop=True,
                )
                st = stages[ib]
                sl = st[:, eh * 512 : (eh + 1) * 512]
                eng = nc.vector
                eng.scalar_tensor_tensor(
                    sl, ps, 1.0, sl, mybir.AluOpType.mult, mybir.AluOpType.add
                )
                nc.sync.dma_start(
                    out[ib * P : (ib + 1) * P, eh * 512 : (eh + 1) * 512], sl
                )

            cx_tiles = {}
            at_tiles = {}

            def scores_exp(s, i2):
                q, h = (s % 8) // 2, 2 * (s // 8) + (s % 2)
                c, r0 = h // 2, (h % 2) * DK
                sc = sc_ps.tile([P, 1024], F32, tag="sc", name="sc")
                for jj in range(2):
                    jc = 2 * i2 + jj
                    nc.tensor.matmul(
                        sc[:, jj * 512 : (jj + 1) * 512],
                        KT[r0 : r0 + DK, c, jc * P : (jc + 1) * P],
                        QT[r0 : r0 + DK, c, q * 512 : (q + 1) * 512],
                        start=True,
                        stop=True,
                    )
                at = at_p.tile([P, 1024], FP16, tag="at", name=f"at{s}_{i2}")
                nc.scalar.activation(at, sc, EXP, scale=0.125)
                at_tiles[(s, i2)] = at

            def ctx_mm(s, i2):
                q, h = (s % 8) // 2, 2 * (s // 8) + (s % 2)
                at = at_tiles.pop((s, i2))
                cx = cx_tiles[s]
                for jj in range(2):
                    jc = 2 * i2 + jj
                    for ib in range(4):
                        nc.tensor.matmul(
                            cx[:, ib, :],
                            at[:, jj * 512 + ib * P : jj * 512 + (ib + 1) * P],
                            Vag[:, h, jc, :],
                            start=False,
                            stop=(i2 == 7 and jj == 1),
                            skip_group_check=True,
                        )

            def norm_sweep(s):
                q, h = (s % 8) // 2, 2 * (s // 8) + (s % 2)
                cx = cx_tiles.pop(s)
                second = h % 2 == 1  # both heads of chunk h//2 done
                rc = small_p.tile([P, 4], F32, tag="rc")
                with nc.allow_low_precision("softmax denom"):
                    nc.vector.reciprocal(
                        rc, cx[:, :, DK : DK + 1].rearrange("p a b -> p (a b)")
                    )
                for ib in range(4):
                    nc.vector.tensor_scalar_mul(
                        ctx_sb[:, q * 4 + ib, h * DK : (h + 1) * DK],
                        cx[:, ib, 0:DK],
                        rc[:, ib : ib + 1],
                    )
                    if second and s < 8:
                        nc.sync.dma_start_transpose(
                            ctxT[:, 0, (q * 4 + ib) * P : (q * 4 + ib + 1) * P],
                            ctx_sb[:, q * 4 + ib, 0:P],
                        )
                if second and s >= 8:
                    # c1 transposes via PE+identity: no SP/HWDGE queueing,
                    # so wo pieces unblock sooner.
                    for ib in range(4):
                        gib = q * 4 + ib
                        tp = wo_ps.tile([P, P], FP16, tag="wo", name="tp")
                        nc.tensor.transpose(
                            tp, ctx_sb[:, gib, P : 2 * P], idn_sb
                        )
                        nc.vector.tensor_copy(
                            ctxT[:, 1, gib * P : (gib + 1) * P], tp
                        )

            # ---- weave units: (deadline, ready, cycles, fn) ----
            weaves = []
            # KT c0 eighths 1..7 (e0 in head): deadline iter 2e (pair-0).
            kt0_rdy = [0, 0, 1, 3, 4, 5, 7, 8]
            for e in range(1, 8):
                for part in range(2):
                    weaves.append(
                        (2 * e - 1, kt0_rdy[e], 1024,
                         lambda e=e, part=part: qk_sub(wk_sb, xk_t, KT, 0, e, part))
                    )
            # KT c1 eighths: needed from iter 64; weave in 20..60.
            for e in range(8):
                for part in range(2):
                    weaves.append(
                        (40 + 2 * e, 14 + e, 1024,
                         lambda e=e, part=part: qk_sub(wk_sb, xk_t, KT, 1, e, part))
                    )
            # QT c0 eighths 2..7 (e0,e1 in head): (c0, qi) by iter 16*qi.
            for e in range(2, 8):
                qi = e // 2
                for part in range(2):
                    weaves.append(
                        (16 * qi - 1, {2: 13, 3: 14, 4: 25, 5: 26, 6: 27, 7: 29}[e],
                         1024, lambda e=e, part=part: qk_sub(wq_sb, xq_t, QT, 0, e, part))
                    )
            # QT c1 eighths: by iter 64 + 16*qi; weave 22..60.
            for e in range(8):
                qi = e // 2
                for part in range(2):
                    weaves.append(
                        (63 + 16 * qi, 22 + e if e < 4 else 25 + e, 1024,
                         lambda e=e, part=part: qk_sub(wq_sb, xq_t, QT, 1, e, part))
                    )
            # V half-0 (heads 0,1): V[jc] by ctx of pair-0 at iter 2*(jc//2)+12.
            for jc in range(16):
                rdy = [10, 16, 19, 21][jc // 4]
                weaves.append(
                    (max(2 * (jc // 2) + 11, rdy), rdy, 1024,
                     lambda jc=jc: v_piece(jc, 0), ("v", jc, 0))
                )
            # V half-1 (heads 2,3): needed from iter ~64; weave 24..60.
            for jc in range(16):
                weaves.append(
                    (62 + 2 * (jc // 2), 18 + jc // 2, 1024,
                     lambda jc=jc: v_piece(jc, 1), ("v", jc, 1))
                )
            # wo: quarter q ready after norm of sweep 8+2q+1 (+transposes).
            for q in range(3):
                s_done = 8 + 2 * q + 1
                rdy = cit[(s_done, 7)] + (3 if q < 2 else 6)
                for ib4 in range(4):
                    for eh in range(2):
                        for part in range(2):
                            weaves.append(
                                (min(rdy + 10, 127), rdy, 512,
                                 lambda q=q, ib4=ib4, eh=eh, part=part:
                                     wo_piece(q * 4 + ib4, eh, part))
                            )
            # q3: c0 halves early (ctxT c0 ready after sweep 7 + transposes),
            # c1 halves + add + store in the tail.
            for ib4 in range(4):
                for eh in range(2):
                    weaves.append(
                        (80, cit[(7, 7)] + 4, 512,
                         lambda ib4=ib4, eh=eh: wo_c0_piece(12 + ib4, eh))
                    )
                    weaves.append(
                        (200 + 2 * ib4 + eh, 131, 512,
                         lambda ib4=ib4, eh=eh: wo_c1_piece(12 + ib4, eh))
                    )
            weaves = [w if len(w) == 5 else (*w, None) for w in weaves]
            weaves.sort(key=lambda u: u[0])
            pending = list(weaves)

            def dummies(n):
                """n x 256 dummy rows to keep the PE p-state ramp alive
                while the head waits on input DMAs."""
                dps = wo_ps.tile([P, 512], F32, tag="wo", name="warmmm")
                for _ in range(n):
                    nc.tensor.matmul(
                        dps[:, 0:256], dmy, dmy2, start=True, stop=True
                    )

            with nc.named_scope("head"):
                dmy = dmy_p.tile([P, P], FP16, tag="dmy")
                dmy2 = dmy_p.tile([P, 256], FP16, tag="dmy2")
                nc.vector.memset(dmy, 0.0)
                nc.vector.memset(dmy2, 0.0)
                dummies(13)  # ramp to full clock by ~2.9us
                qk_sub(wq_sb, xq_t, QT, 0, 0)
                dummies(8)  # bridge to xq e1 arrival ~7.7us
                qk_sub(wq_sb, xq_t, QT, 0, 1)
                dummies(4)  # bridge to xk e0 arrival ~9.2us
                qk_sub(wk_sb, xk_t, KT, 0, 0)

            debt = 0
            for t in range(131):
                with nc.named_scope(f"it{t}"):
                    used = 0
                    if t in scores_at:
                        s, i2 = scores_at[t]
                        if i2 == 0:
                            cx_tiles[s] = cx_ps.tile(
                                [P, 4, DK + 1], F32, tag="cx", name=f"cx{s}"
                            )
                            nc.vector.memset(cx_tiles[s], 0.0)
                        scores_exp(s, i2)
                        used += 1024
                    for key in ctx_at.get(t, []):
                        s2k, i22k = key
                        h2k = 2 * (s2k // 8) + (s2k % 2)
                        need = {("v", 2 * i22k, h2k // 2),
                                ("v", 2 * i22k + 1, h2k // 2)}
                        for i in range(len(pending) - 1, -1, -1):
                            if pending[i][4] in need:
                                ent = pending.pop(i)
                                ent[3]()
                                used += ent[2]
                        ctx_mm(*key)
                        used += 520
                        if key in norm_after:
                            norm_sweep(norm_after[key])
                    if t >= 128:
                        used = -(10**9)
                    while pending:
                        idx = None
                        for i, (dl, rdy, cyc, fn, wkey) in enumerate(pending):
                            debt_after = max(0, debt + used + cyc - CAP)
                            if rdy <= t and (debt_after <= MAX_DEBT or dl <= t):
                                idx = i
                                break
                        if idx is None:
                            break
                        dl, rdy, cyc, fn, wkey = pending.pop(idx)
                        fn()
                        used += cyc
                    debt = max(0, debt + used - CAP) if t < 128 else 0
            with nc.named_scope("tail"):
                for ent in pending:
                    ent[3]()
    nc.compile()
    return nc


def get_nc():
    if not _NC_CACHE:
        _NC_CACHE.append(_build_nc())
    return _NC_CACHE[0]


def kernel(query, key, value, mask, Wq, Wk, Wv, Wo, **_run_kwargs):
    query = np.asarray(query, np.float32)
    key = np.asarray(key, np.float32)
    value = np.asarray(value, np.float32)
    Wq = np.asarray(Wq, np.float32)
    Wk = np.asarray(Wk, np.float32)
    Wv = np.asarray(Wv, np.float32)
    Wo = np.asarray(Wo, np.float32)

    nc = get_nc()
    f16 = np.float16
    in_maps = []
    for b in range(2):
        xqTb = np.ascontiguousarray(query[b].T).astype(f16)
        xkTb = np.ascontiguousarray(key[b].T).astype(f16)
        xvTb = np.ascontiguousarray(value[b].T).astype(f16)
        for g in range(4):
            c0 = g * DKV
            in_maps.append(
                {
                    "xqT": xqTb,
                    "xkT": xkTb,
                    "xvT": xvTb,
                    "wq": np.ascontiguousarray(Wq[:, c0 : c0 + DKV]).astype(f16),
                    "wk": np.ascontiguousarray(Wk[:, c0 : c0 + DKV]).astype(f16),
                    "wv": np.ascontiguousarray(Wv[:, c0 : c0 + DKV]).astype(f16),
                    "wo": np.ascontiguousarray(Wo[c0 : c0 + DKV, :]).astype(f16),
                    "idn": np.eye(P, dtype=f16),
                }
            )
    res = run_bass_kernel_spmd(nc, in_maps, core_ids=list(range(8)), **_run_kwargs)
    outs = [r["out"] for r in res.results]
    full = np.stack(
        [
            outs[0] + outs[1] + outs[2] + outs[3],
            outs[4] + outs[5] + outs[6] + outs[7],
        ]
    ).astype(np.float32)
    if _run_kwargs:
        return full, res
    return full


# revision 8
# speedup vs baseline: 1.0432x; 1.0027x over previous
"""Multi-head attention TRN2 kernel, v2.

Full inputs -> 8-core shard (batch x head-group) -> Bass/Tile kernel -> host
gather+reduce.  Problem: B=2, S=2048, D=1024, H=16, Dk=64, fp32, mask=0.

Core c = b*4 + g handles batch b, heads 4g..4g+3.  All intermediates fp16.

Engine plan (cost-model driven):
  PE    projections (full-128 contraction), scoresT (K=64), ctx with the
        attn chunk STATIONARY and V MOVING (65 rows incl. a ones column so
        the softmax denominator rides along), Wo.
  Act   exp only: one [128,1024] activation per iteration (two j-chunks
        side by side), scale=0.125 folded in.  This stream (~134us busy)
        is the bottleneck; everything else hides under it.
  DVE   psum->sbuf copies + reciprocals.
  Pool  softmax normalize + half the psum->sbuf drains.
  DMA   input stream, ctx transposes via the XBAR, output writes.

Sweeps (i-quarter q, head h) are ordered h-pair-outer: heads 0/1 for all
quarters first (sweeps 0..7 = q0h0,q0h1,q1h0,...), then heads 2/3
(sweeps 8..15).  KT/QT chunk-0 thus feeds the first 8 sweeps and chunk-1
streams in later.  Sweeps 0 and 1 are jc-interleaved into one 16-slot
stretch so the fresh-KT demand rate stays under the HBM stream rate.
ctx matmuls run a few iterations behind their exp (explicit CIT map);
V/QT/KT/Wo pieces fill PE slack via a deadline-greedy budgeter.
PSUM: scores ring 2x2 banks, ctx accums 2x1 bank, wo/proj ring 2x1 bank.
"""

import sys

import numpy as np

try:
    import concourse.bass as bass  # noqa: F401
except ImportError:  # harness runs from a bare directory
    sys.path.insert(0, "/opt/trn_rl_repo")
    import concourse.bass as bass  # noqa: F401

import concourse.tile as tile
from concourse import bacc, mybir
from concourse.bass_utils import run_bass_kernel_spmd

S = 2048
D = 1024
HG = 4  # heads per core
DK = 64
DKV = HG * DK  # 256
P = 128
F32 = mybir.dt.float32
FP16 = mybir.dt.float16
EXP = mybir.ActivationFunctionType.Exp

_NC_CACHE = []

AT_RING = 15
CAP = 2400  # PE cycles per Act period less per-instr overheads
MAX_DEBT = 1200  # PE-behind allowance absorbed by the sc ring


def _iteration_maps():
    """SIT: (s, i2) -> scores iteration; CIT: ctx iteration; both 0-based
    over 128 slots.  Sweeps 0/1 are interleaved over slots 0..15."""
    sit = {}
    for s in range(16):
        for i2 in range(8):
            if s < 2:
                sit[(s, i2)] = 2 * i2 + s
            else:
                sit[(s, i2)] = 8 * s + i2

    lag = {0: 13, 1: 13, 2: 13, 3: 12, 4: 11, 5: 10, 6: 9, 7: 8,
           8: 7, 9: 6, 10: 5, 11: 4, 12: 3, 13: 2, 14: 2, 15: 1}
    cit = {}
    for s in range(16):
        L = lag.get(s, 2)
        for i2 in range(8):
            cit[(s, i2)] = sit[(s, i2)] + L
    return sit, cit


def _build_nc():
    nc = bacc.Bacc("TRN2", target_bir_lowering=False, debug=False)
    xqT = nc.dram_tensor("xqT", [D, S], FP16, kind="ExternalInput")
    xkT = nc.dram_tensor("xkT", [D, S], FP16, kind="ExternalInput")
    xvT = nc.dram_tensor("xvT", [D, S], FP16, kind="ExternalInput")
    wq = nc.dram_tensor("wq", [D, DKV], FP16, kind="ExternalInput")
    wk = nc.dram_tensor("wk", [D, DKV], FP16, kind="ExternalInput")
    wv = nc.dram_tensor("wv", [D, DKV], FP16, kind="ExternalInput")
    wo = nc.dram_tensor("wo", [DKV, D], FP16, kind="ExternalInput")
    idn = nc.dram_tensor("idn", [P, P], FP16, kind="ExternalInput")
    out = nc.dram_tensor("out", [S, D], F32, kind="ExternalOutput")

    sit, cit = _iteration_maps()
    scores_at = {}  # iter -> (s, i2)
    for k, t in sit.items():
        scores_at[t] = k
    ctx_at = {}
    for (s, i2), t in cit.items():
        ctx_at.setdefault(t, []).append((s, i2))
        ctx_at[t].sort(key=lambda k: (cit[k], k))
    norm_after = {}  # (s, i2) -> s to normalize right after that ctx
    for s in range(16):
        norm_after[(s, 7)] = s

    with tile.TileContext(nc) as tc:
        with (
            tc.tile_pool(name="persist", bufs=1) as persist,
            tc.tile_pool(name="xk_p", bufs=8) as xk_p,
            tc.tile_pool(name="xq_p", bufs=8) as xq_p,
            tc.tile_pool(name="xv_p", bufs=8) as xv_p,
            tc.tile_pool(name="at_p", bufs=AT_RING) as at_p,
            tc.tile_pool(name="stage", bufs=2) as stage_p,
            tc.tile_pool(name="q3st", bufs=4) as q3st_p,
            tc.tile_pool(name="small", bufs=4) as small_p,
            tc.tile_pool(name="dmy", bufs=1) as dmy_p,
            tc.tile_pool(name="sc_ps", bufs=2, space="PSUM") as sc_ps,
            tc.tile_pool(name="cx_ps", bufs=2, space="PSUM") as cx_ps,
            tc.tile_pool(name="wo_ps", bufs=2, space="PSUM") as wo_ps,
        ):
            # ---- persistent SBUF ----
            QT = persist.tile([P, 2, S], FP16)  # [(h%2)*64+dk, h//2, i]
            KT = persist.tile([P, 2, S], FP16)
            Vag = persist.tile([P, HG, 16, DK + 1], FP16)  # [j%128, h, jc, dk|1]
            ctx_sb = persist.tile([P, 16, DKV], FP16)  # [i%128, ib, dkv]
            ctxT = persist.tile([P, 2, S], FP16)  # [dkv%128, dkv//128, i]
            wq_sb = persist.tile([P, 8, DKV], FP16)
            wk_sb = persist.tile([P, 8, DKV], FP16)
            wv_sb = persist.tile([P, 8, DKV], FP16)
            wo_sb = persist.tile([P, 2, D], FP16)
            idn_sb = persist.tile([P, P], FP16)

            # ---- head: exp-table warm + ones column ----
            warm = small_p.tile([P, 8], F32, tag="warm")
            nc.vector.memset(warm[0:1, :], 0.0)
            nc.scalar.activation(warm[0:1, :], warm[0:1, :], EXP, scale=0.0)
            nc.vector.memset(Vag[:, :, :, DK : DK + 1], 1.0)

            xk_t, xq_t, xv_t = {}, {}, {}

            def load_x(pool, store, xT, idx, width, name):
                t = pool.tile([P, 8, width], FP16, tag="x", name=f"{name}{idx}")
                nc.sync.dma_start(
                    t,
                    xT.rearrange("(ko p) i -> p ko i", p=P)[
                        :, :, idx * width : (idx + 1) * width
                    ],
                )
                store[idx] = t

            # DMA issue order = arrival priority (in-order SP queue).
            nc.sync.dma_start(wq_sb, wq.rearrange("(ko p) n -> p ko n", p=P))
            load_x(xq_p, xq_t, xqT, 0, 256, "xq")
            nc.sync.dma_start(wk_sb, wk.rearrange("(ko p) n -> p ko n", p=P))
            load_x(xq_p, xq_t, xqT, 1, 256, "xq")
            for e in range(0, 8):
                load_x(xk_p, xk_t, xkT, e, 256, "xk")
            nc.sync.dma_start(wv_sb, wv.rearrange("(ko p) n -> p ko n", p=P))
            load_x(xv_p, xv_t, xvT, 0, 256, "xv")
            load_x(xv_p, xv_t, xvT, 1, 256, "xv")
            load_x(xq_p, xq_t, xqT, 2, 256, "xq")
            load_x(xq_p, xq_t, xqT, 3, 256, "xq")
            for e in range(2, 8):
                load_x(xv_p, xv_t, xvT, e, 256, "xv")
            load_x(xq_p, xq_t, xqT, 4, 256, "xq")
            load_x(xq_p, xq_t, xqT, 5, 256, "xq")
            load_x(xq_p, xq_t, xqT, 6, 256, "xq")
            load_x(xq_p, xq_t, xqT, 7, 256, "xq")
            nc.sync.dma_start(wo_sb, wo.rearrange("(c p) n -> p c n", p=P))
            nc.sync.dma_start(idn_sb, idn[:, :])

            # ---- piece emitters (all 256-wide sub-pieces for KT/QT) ----
            qk_open = {}

            def qk_sub(w_sb, x_t, OUT, c, e, part=2):
                """OUT[:, c, e*256:(e+1)*256] from x eighth e (2048 cycles);
                part=0/1 emit one 1024-cycle half of the k-accumulation."""
                key = (id(w_sb), c, e)
                if part in (0, 2):
                    qk_open[key] = wo_ps.tile([P, 512], F32, tag="wo", name="qk")
                ps = qk_open[key]
                ks = range(8) if part == 2 else range(4 * part, 4 * part + 4)
                for k in ks:
                    nc.tensor.matmul(
                        ps[:, 0:256],
                        w_sb[:, k, c * P : (c + 1) * P],
                        x_t[e][:, k, :],
                        start=(k == 0),
                        stop=(k == 7),
                    )
                if part in (1, 2):
                    nc.vector.tensor_copy(
                        OUT[:, c, e * 256 : (e + 1) * 256], ps[:, 0:256]
                    )
                    del qk_open[key]

            def v_piece(jc, half):
                """Vag[:, 2*half:2*half+2, jc, 0:64] (1024 cycles)."""
                ps = wo_ps.tile([P, 512], F32, tag="wo", name="v")
                xt = xv_t[jc // 2]
                j0 = (jc % 2) * P
                for k in range(8):
                    nc.tensor.matmul(
                        ps[:, 0:P],
                        xt[:, k, j0 : j0 + P],
                        wv_sb[:, k, half * P : (half + 1) * P],
                        start=(k == 0),
                        stop=(k == 7),
                    )
                nc.vector.tensor_copy(
                    Vag[:, 2 * half : 2 * half + 2, jc, 0:DK],
                    ps[:, 0:P].rearrange("p (h d) -> p h d", h=2),
                )

            stages = {}
            wo_done = {}

            wo_open = {}

            def wo_piece(ib, eh, part=2):
                if ib not in stages:
                    stages[ib] = stage_p.tile([P, D], F32, tag="st", name=f"st{ib}")
                if part in (0, 2):
                    wo_open[(ib, eh)] = wo_ps.tile([P, 512], F32, tag="wo",
                                                   name="woo")
                ps = wo_open[(ib, eh)]
                cs = range(2) if part == 2 else [part]
                for c in cs:
                    nc.tensor.matmul(
                        ps,
                        ctxT[:, c, ib * P : (ib + 1) * P],
                        wo_sb[:, c, eh * 512 : (eh + 1) * 512],
                        start=(c == 0),
                        stop=(c == 1),
                    )
                if part in (1, 2):
                    del wo_open[(ib, eh)]
                    st = stages[ib]
                    nc.vector.tensor_copy(st[:, eh * 512 : (eh + 1) * 512], ps)
                    done = wo_done.setdefault(ib, set())
                    done.add(eh)
                    if done == {0, 1}:
                        nc.sync.dma_start(out[ib * P : (ib + 1) * P, :], st)

            # Last quarter: wo split into an early c0 half (staged to SBUF)
            # and a small tail c1 half + add, to shorten the tail chain.
            def wo_c0_piece(ib, eh):
                if ib not in stages:
                    stages[ib] = q3st_p.tile([P, D], F32, tag="q3st",
                                             name=f"q3st{ib}")
                ps = wo_ps.tile([P, 512], F32, tag="wo", name="woa")
                nc.tensor.matmul(
                    ps,
                    ctxT[:, 0, ib * P : (ib + 1) * P],
                    wo_sb[:, 0, eh * 512 : (eh + 1) * 512],
                    start=True,
                    stop=True,
                )
                eng = nc.vector
                eng.tensor_copy(stages[ib][:, eh * 512 : (eh + 1) * 512], ps)

            wo_c1_ps = {}

            def wo_c1_piece(ib, eh):
                if ib not in wo_c1_ps:
                    wo_c1_ps[ib] = sc_ps.tile([P, 1024], F32, tag="sc", name="wob")
                ps = wo_c1_ps[ib][:, eh * 512 : (eh + 1) * 512]
                nc.tensor.matmul(
                    ps,
                    ctxT[:, 1, ib * P : (ib + 1) * P],
                    wo_sb[:, 1, eh * 512 : (eh + 1) * 512],
                    start=True,
                    stop=True,
                )
                st = stages[ib]
                sl = st[:, eh * 512 : (eh + 1) * 512]
                eng = nc.vector
                eng.scalar_tensor_tensor(
                    sl, ps, 1.0, sl, mybir.AluOpType.mult, mybir.AluOpType.add
                )
                nc.sync.dma_start(
                    out[ib * P : (ib + 1) * P, eh * 512 : (eh + 1) * 512], sl
                )

            cx_tiles = {}
            at_tiles = {}

            def scores_exp(s, i2):
                q, h = (s % 8) // 2, 2 * (s // 8) + (s % 2)
                c, r0 = h // 2, (h % 2) * DK
                sc = sc_ps.tile([P, 1024], F32, tag="sc", name="sc")
                for jj in range(2):
                    jc = 2 * i2 + jj
                    nc.tensor.matmul(
                        sc[:, jj * 512 : (jj + 1) * 512],
                        KT[r0 : r0 + DK, c, jc * P : (jc + 1) * P],
                        QT[r0 : r0 + DK, c, q * 512 : (q + 1) * 512],
                        start=True,
                        stop=True,
                    )
                at = at_p.tile([P, 1024], FP16, tag="at", name=f"at{s}_{i2}")
                nc.scalar.activation(at, sc, EXP, scale=0.125)
                at_tiles[(s, i2)] = at

            def ctx_mm(s, i2):
                q, h = (s % 8) // 2, 2 * (s // 8) + (s % 2)
                at = at_tiles.pop((s, i2))
                cx = cx_tiles[s]
                for jj in range(2):
                    jc = 2 * i2 + jj
                    for ib in range(4):
                        nc.tensor.matmul(
                            cx[:, ib, :],
                            at[:, jj * 512 + ib * P : jj * 512 + (ib + 1) * P],
                            Vag[:, h, jc, :],
                            start=False,
                            stop=(i2 == 7 and jj == 1),
                            skip_group_check=True,
                        )

            def norm_sweep(s):
                q, h = (s % 8) // 2, 2 * (s // 8) + (s % 2)
                cx = cx_tiles.pop(s)
                second = h % 2 == 1  # both heads of chunk h//2 done
                rc = small_p.tile([P, 4], F32, tag="rc")
                with nc.allow_low_precision("softmax denom"):
                    nc.vector.reciprocal(
                        rc, cx[:, :, DK : DK + 1].rearrange("p a b -> p (a b)")
                    )
                for ib in range(4):
                    nc.vector.tensor_scalar_mul(
                        ctx_sb[:, q * 4 + ib, h * DK : (h + 1) * DK],
                        cx[:, ib, 0:DK],
                        rc[:, ib : ib + 1],
                    )
                    if second and s < 8:
                        nc.sync.dma_start_transpose(
                            ctxT[:, 0, (q * 4 + ib) * P : (q * 4 + ib + 1) * P],
                            ctx_sb[:, q * 4 + ib, 0:P],
                        )
                if second and s >= 8:
                    # c1 transposes via PE+identity: no SP/HWDGE queueing,
                    # so wo pieces unblock sooner.
                    for ib in range(4):
                        gib = q * 4 + ib
                        tp = wo_ps.tile([P, P], FP16, tag="wo", name="tp")
                        nc.tensor.transpose(
                            tp, ctx_sb[:, gib, P : 2 * P], idn_sb
                        )
                        nc.vector.tensor_copy(
                            ctxT[:, 1, gib * P : (gib + 1) * P], tp
                        )

            # ---- weave units: (deadline, ready, cycles, fn) ----
            weaves = []
            # KT c0 eighths 1..7 (e0 in head): deadline iter 2e (pair-0).
            kt0_rdy = [0, 0, 1, 3, 4, 5, 7, 8]
            for e in range(1, 8):
                for part in range(2):
                    weaves.append(
                        (2 * e - 1, kt0_rdy[e], 1024,
                         lambda e=e, part=part: qk_sub(wk_sb, xk_t, KT, 0, e, part))
                    )
            # KT c1 eighths: needed from iter 64; weave in 20..60.
            for e in range(8):
                for part in range(2):
                    weaves.append(
                        (40 + 2 * e, 14 + e, 1024,
                         lambda e=e, part=part: qk_sub(wk_sb, xk_t, KT, 1, e, part))
                    )
            # QT c0 eighths 2..7 (e0,e1 in head): (c0, qi) by iter 16*qi.
            for e in range(2, 8):
                qi = e // 2
                for part in range(2):
                    weaves.append(
                        (16 * qi - 1, {2: 13, 3: 14, 4: 25, 5: 26, 6: 27, 7: 29}[e],
                         1024, lambda e=e, part=part: qk_sub(wq_sb, xq_t, QT, 0, e, part))
                    )
            # QT c1 eighths: by iter 64 + 16*qi; weave 22..60.
            for e in range(8):
                qi = e // 2
                for part in range(2):
                    weaves.append(
                        (63 + 16 * qi, 22 + e if e < 4 else 25 + e, 1024,
                         lambda e=e, part=part: qk_sub(wq_sb, xq_t, QT, 1, e, part))
                    )
            # V half-0 (heads 0,1): V[jc] by ctx of pair-0 at iter 2*(jc//2)+12.
            for jc in range(16):
                rdy = [10, 11, 12, 14, 15, 17, 18, 20][jc // 2]
                weaves.append(
                    (max(2 * (jc // 2) + 11, rdy), rdy, 1024,
                     lambda jc=jc: v_piece(jc, 0), ("v", jc, 0))
                )
            # V half-1 (heads 2,3): needed from iter ~64; weave 24..60.
            for jc in range(16):
                weaves.append(
                    (62 + 2 * (jc // 2), 18 + jc // 2, 1024,
                     lambda jc=jc: v_piece(jc, 1), ("v", jc, 1))
                )
            # wo: quarter q ready after norm of sweep 8+2q+1 (+transposes).
            for q in range(3):
                s_done = 8 + 2 * q + 1
                rdy = cit[(s_done, 7)] + (3 if q < 2 else 6)
                for ib4 in range(4):
                    for eh in range(2):
                        for part in range(2):
                            weaves.append(
                                (min(rdy + 10, 127), rdy, 512,
                                 lambda q=q, ib4=ib4, eh=eh, part=part:
                                     wo_piece(q * 4 + ib4, eh, part))
                            )
            # q3: c0 halves early (ctxT c0 ready after sweep 7 + transposes),
            # c1 halves + add + store in the tail.
            for ib4 in range(4):
                for eh in range(2):
                    weaves.append(
                        (80, cit[(7, 7)] + 4, 512,
                         lambda ib4=ib4, eh=eh: wo_c0_piece(12 + ib4, eh))
                    )
                    weaves.append(
                        (200 + 2 * ib4 + eh, 131, 512,
                         lambda ib4=ib4, eh=eh: wo_c1_piece(12 + ib4, eh))
                    )
            weaves = [w if len(w) == 5 else (*w, None) for w in weaves]
            weaves.sort(key=lambda u: u[0])
            pending = list(weaves)

            def dummies(n):
                """n x 256 dummy rows to keep the PE p-state ramp alive
                while the head waits on input DMAs."""
                dps = wo_ps.tile([P, 512], F32, tag="wo", name="warmmm")
                for _ in range(n):
                    nc.tensor.matmul(
                        dps[:, 0:256], dmy, dmy2, start=True, stop=True
                    )

            with nc.named_scope("head"):
                dmy = dmy_p.tile([P, P], FP16, tag="dmy")
                dmy2 = dmy_p.tile([P, 256], FP16, tag="dmy2")
                nc.vector.memset(dmy, 0.0)
                nc.vector.memset(dmy2, 0.0)
                dummies(13)  # ramp to full clock by ~2.9us
                qk_sub(wq_sb, xq_t, QT, 0, 0)
                dummies(8)  # bridge to xq e1 arrival ~7.7us
                qk_sub(wq_sb, xq_t, QT, 0, 1)
                dummies(4)  # bridge to xk e0 arrival ~9.2us
                qk_sub(wk_sb, xk_t, KT, 0, 0)

            debt = 0
            for t in range(131):
                with nc.named_scope(f"it{t}"):
                    used = 0
                    if t in scores_at:
                        s, i2 = scores_at[t]
                        if i2 == 0:
                            cx_tiles[s] = cx_ps.tile(
                                [P, 4, DK + 1], F32, tag="cx", name=f"cx{s}"
                            )
                            nc.vector.memset(cx_tiles[s], 0.0)
                        scores_exp(s, i2)
                        used += 1024
                    for key in ctx_at.get(t, []):
                        s2k, i22k = key
                        h2k = 2 * (s2k // 8) + (s2k % 2)
                        need = {("v", 2 * i22k, h2k // 2),
                                ("v", 2 * i22k + 1, h2k // 2)}
                        for i in range(len(pending) - 1, -1, -1):
                            if pending[i][4] in need:
                                ent = pending.pop(i)
                                ent[3]()
                                used += ent[2]
                        ctx_mm(*key)
                        used += 520
                        if key in norm_after:
                            norm_sweep(norm_after[key])
                    if t >= 128:
                        used = -(10**9)
                    while pending:
                        idx = None
                        for i, (dl, rdy, cyc, fn, wkey) in enumerate(pending):
                            debt_after = max(0, debt + used + cyc - CAP)
                            if rdy <= t and (debt_after <= MAX_DEBT or dl <= t):
                                idx = i
                                break
                        if idx is None:
                            break
                        dl, rdy, cyc, fn, wkey = pending.pop(idx)
                        fn()
                        used += cyc
                    debt = max(0, debt + used - CAP) if t < 128 else 0
            with nc.named_scope("tail"):
                for ent in pending:
                    ent[3]()
    nc.compile()
    return nc


def get_nc():
    if not _NC_CACHE:
        _NC_CACHE.append(_build_nc())
    return _NC_CACHE[0]


def kernel(query, key, value, mask, Wq, Wk, Wv, Wo, **_run_kwargs):
    query = np.asarray(query, np.float32)
    key = np.asarray(key, np.float32)
    value = np.asarray(value, np.float32)
    Wq = np.asarray(Wq, np.float32)
    Wk = np.asarray(Wk, np.float32)
    Wv = np.asarray(Wv, np.float32)
    Wo = np.asarray(Wo, np.float32)

    nc = get_nc()
    f16 = np.float16
    in_maps = []
    for b in range(2):
        xqTb = np.ascontiguousarray(query[b].T).astype(f16)
        xkTb = np.ascontiguousarray(key[b].T).astype(f16)
        xvTb = np.ascontiguousarray(value[b].T).astype(f16)
        for g in range(4):
            c0 = g * DKV
            in_maps.append(
                {
                    "xqT": xqTb,
                    "xkT": xkTb,
                    "xvT": xvTb,
                    "wq": np.ascontiguousarray(Wq[:, c0 : c0 + DKV]).astype(f16),
                    "wk": np.ascontiguousarray(Wk[:, c0 : c0 + DKV]).astype(f16),
                    "wv": np.ascontiguousarray(Wv[:, c0 : c0 + DKV]).astype(f16),
                    "wo": np.ascontiguousarray(Wo[c0 : c0 + DKV, :]).astype(f16),
                    "idn": np.eye(P, dtype=f16),
                }
            )
    res = run_bass_kernel_spmd(nc, in_maps, core_ids=list(range(8)), **_run_kwargs)
    outs = [r["out"] for r in res.results]
    full = np.stack(
        [
            outs[0] + outs[1] + outs[2] + outs[3],
            outs[4] + outs[5] + outs[6] + outs[7],
        ]
    ).astype(np.float32)
    if _run_kwargs:
        return full, res
    return full
